# revision 15
# baseline (speedup 1.0000x reference)
"""DGL-GAT subgraph encoder kernel for 8 Trainium2 NeuronCores.

With IN_FEATS=1 the GATConv collapses to per-node scalars:
  feat[n,h,d] = f[n]*W1[h,d];  el[n,h] = f[n]*cl[h];  er[n,h] = f[n]*cr[h]
  w[e,h] = exp(lrelu(f[src]*cl[h] + f[dst]*cr[h]))   (softmax max-shift cancels
  in the num/denom ratio; exponents stay < ~25 so no overflow)
  denom[n,h] = seg_sum_dst(w);  num[n,h] = seg_sum_dst(w * f[src])
  s[n,h] = num/denom;  sbar[h] = mean_n s
  out = (sbar[h]*W1[h,:] + bias_gat) @ fc_W + fc_b     (tiny, done on host)

Sharding: core k owns dst nodes [k*12500, (k+1)*12500) and all edges into
them.  Edges are dst-sorted into window-pure 128-edge columns (32-node
one-hot windows, uniform capacity so all cores share one program).  The
device computes per-edge w and w*fs (DVE/ACT) and the two segment sums via
PE matmuls  V[128e,8]^T x onehot[128e,32] accumulated in [8,512] PSUM
blocks; per-core partial (denom,num) tables return to the host, which does
the 100K-node ratio/mean and the final 256x128 projection.
"""
import numpy as np
import ml_dtypes
import concourse.bass as bass
import concourse.tile as tile
from concourse import bacc, mybir, bass_utils

WIN = 8           # nodes per one-hot window (matmul N)
BLK = 512         # nodes per psum block
P = 128           # edges per column
CHK = 128         # columns per onehot chunk
CCH = 512         # columns per compute/load chunk
NCORES = 8

BF16 = ml_dtypes.bfloat16


def _plan(n_nodes, nwin_max):
    nodes_pc = -(-n_nodes // NCORES)
    ncw = 1
    C = -(-(nwin_max * ncw) // CHK) * CHK
    nblk = ((C - 1) // ncw) // (BLK // WIN) + 1
    return dict(nodes_pc=nodes_pc, nwin=nwin_max, ncw=ncw, C=C, nblk=nblk)


def _pack_windows(deg):
    """Greedy sequential packing: nodes (in order) into windows of <=WIN nodes
    and <=ncw*P edges.  Returns per-node window id and within-window slot."""
    cap = 1 * P
    nodewin = np.empty(len(deg), dtype=np.int64)
    nodeslot = np.empty(len(deg), dtype=np.int64)
    w = nn = ee = 0
    for i, dg in enumerate(deg):
        if nn >= WIN or ee + dg > cap:
            w += 1; nn = 0; ee = 0
        nodewin[i] = w
        nodeslot[i] = nn
        nn += 1; ee += dg
    return nodewin, nodeslot, w + 1


def _host_prep_core(f, src_c, dst_c, lo, pl, nodewin, nodeslot):
    ncw, C = pl["ncw"], pl["C"]
    o = np.argsort(dst_c, kind="stable")
    s_c, d_c = src_c[o], dst_c[o]
    nloc = d_c - lo
    win = nodewin[nloc]
    idl = nodeslot[nloc]
    starts = np.searchsorted(win, np.arange(pl["nwin"]))
    rank = np.arange(len(win)) - starts[win]
    cap = ncw * P
    assert rank.max(initial=0) < cap, "window capacity overflow"
    flat = win * cap + rank

    def scatter(vals, fill, dt):
        a = np.full(C * P, fill, dtype=np.float32)
        a[flat] = vals
        return np.ascontiguousarray(a.reshape(C, P).T).astype(dt)

    return dict(fs=scatter(f[s_c], 0.0, np.float32),
                fd=scatter(f[d_c], 0.0, np.float32),
                ids=scatter(idl.astype(np.float32), -1.0, BF16))


def _build_program(pl):
    C, ncw, nblk = pl["C"], pl["ncw"], pl["nblk"]
    nc = bacc.Bacc("TRN2", target_bir_lowering=False, debug=False,
                   enable_asserts=False, num_devices=NCORES)
    bf = mybir.dt.bfloat16
    f32 = mybir.dt.float32

    fs_d = nc.dram_tensor("fs", [P, C], f32, kind="ExternalInput").ap()
    fd_d = nc.dram_tensor("fd", [P, C], f32, kind="ExternalInput").ap()
    ids_d = nc.dram_tensor("ids", [P, C], bf, kind="ExternalInput").ap()
    prm_d = nc.dram_tensor("prm", [P, 8], f32, kind="ExternalInput").ap()
    nsup = -(-nblk // 3)
    acc_d = nc.dram_tensor("acc", [P, nsup * BLK], f32, kind="ExternalOutput").ap()
    wpb = BLK // WIN

    with tile.TileContext(nc) as tc:
        with tc.tile_pool(name="consts", bufs=1) as cpool, \
             tc.tile_pool(name="io", bufs=4) as io, \
             tc.tile_pool(name="work", bufs=3) as work, \
             tc.tile_pool(name="ohp", bufs=6) as ohp, \
             tc.tile_pool(name="flp", bufs=3) as flp, \
             tc.tile_pool(name="psum", bufs=4, space="PSUM") as psum_p:
            def flush(sup, ps):
                st = flp.tile([P, BLK], f32, tag="fl")
                nc.vector.tensor_copy(st[:], ps[:])
                nc.sync.dma_start(acc_d[:, sup * BLK:(sup + 1) * BLK], st[:])

            prm = cpool.tile([P, 8], f32, name="prm_s")
            nc.sync.dma_start(prm[:], prm_d)
            iota = cpool.tile([P, WIN], mybir.dt.int16, name="iota_s")
            nc.gpsimd.iota(iota[:], pattern=[[1, WIN]], base=0, channel_multiplier=0)
            iotab = cpool.tile([P, WIN], bf, name="iotab_s")
            nc.vector.tensor_copy(iotab[:], iota[:])

            psum_t, cur_blk = None, -1
            chunks = []
            c0x = 0
            while c0x < C:
                chunks.append((c0x, min(CCH, C - c0x)))
                c0x += CCH
            for c0, CL in chunks:
                fst = io.tile([P, CCH], f32, tag="fs")
                fdt = io.tile([P, CCH], f32, tag="fd")
                idst = io.tile([P, CCH], bf, tag="ids")
                fs = fst[:, :CL]; fd = fdt[:, :CL]; ids = idst[:, :CL]
                nc.sync.dma_start(fs, fs_d[:, c0:c0 + CL])
                nc.sync.dma_start(fd, fd_d[:, c0:c0 + CL])
                nc.sync.dma_start(ids, ids_d[:, c0:c0 + CL])

                vi = work.tile([P, 8 * CCH], bf, tag="vi")
                vi3 = vi[:].rearrange("p (v c) -> p v c", v=8)[:, :, :CL]
                t1 = work.tile([P, CCH], f32, tag="t1", name="t1t")[:, :CL]
                z = work.tile([P, CCH], f32, tag="z", name="zt")[:, :CL]
                e1 = work.tile([P, CCH], bf, tag="e1", name="e1t")[:, :CL]
                e2 = work.tile([P, CCH], bf, tag="e2", name="e2t")[:, :CL]
                fsb = work.tile([P, CCH], bf, tag="fsb", name="fsbt")[:, :CL]
                nc.vector.tensor_copy(fsb, fs)
                for h in range(4):
                    nc.vector.tensor_scalar_mul(t1, fd, prm[:, 4 + h:5 + h])
                    nc.vector.scalar_tensor_tensor(
                        out=z, in0=fs, scalar=prm[:, h:h + 1], in1=t1,
                        op0=mybir.AluOpType.mult, op1=mybir.AluOpType.add)
                    nc.scalar.activation(e1, z, mybir.ActivationFunctionType.Exp)
                    nc.scalar.activation(e2, z, mybir.ActivationFunctionType.Exp,
                                         scale=0.2)
                    nc.vector.tensor_tensor(out=vi3[:, h, :], in0=e1, in1=e2,
                                            op=mybir.AluOpType.max)
                    nc.vector.tensor_mul(vi3[:, 4 + h, :], vi3[:, h, :], fsb)

                for ch in range(CL // CHK):
                    t0 = c0 + ch * CHK
                    oh = ohp.tile([P, CHK * WIN], bf, tag="oh")
                    nc.vector.tensor_tensor(
                        out=oh[:].rearrange("p (c w) -> p c w", w=WIN),
                        in0=ids[:, ch * CHK:(ch + 1) * CHK].unsqueeze(-1)
                            .to_broadcast([P, CHK, WIN]),
                        in1=iotab[:].unsqueeze(1).to_broadcast([P, CHK, WIN]),
                        op=mybir.AluOpType.is_equal)
                    for tl in range(CHK):
                        t = t0 + tl
                        w = t // ncw
                        b = w // wpb
                        sup = b // 3
                        if sup != cur_blk:
                            if psum_t is not None:
                                flush(cur_blk, psum_t)
                            psum_t = psum_p.tile([P, BLK], f32, tag="ps")
                            cur_blk = sup
                        wl = w % wpb
                        po = 32 * (b % 3)
                        nc.tensor.matmul(
                            out=psum_t[po:po + 8, wl * WIN:(wl + 1) * WIN],
                            lhsT=vi3[:, :, t - c0],
                            rhs=oh[:, tl * WIN:(tl + 1) * WIN],
                            start=(t % ncw == 0), stop=(t % ncw == ncw - 1))
            flush(cur_blk, psum_t)
    nc.compile()
    return nc


def kernel(features, W, attn_l, attn_r, bias_gat, fc_W, fc_b, src, dst):
    f = np.asarray(features, dtype=np.float32)[:, 0]
    src = np.asarray(src)
    dst = np.asarray(dst)
    N = f.shape[0]
    H, D = np.asarray(attn_l).shape

    nodes_pc = -(-N // NCORES)
    packs = []
    for k in range(NCORES):
        lo = k * nodes_pc
        npc = min(nodes_pc, N - lo)
        deg = np.bincount(dst[(dst >= lo) & (dst < lo + npc)] - lo, minlength=npc)
        packs.append(_pack_windows(deg))
    pl = _plan(N, max(pk[2] for pk in packs))

    W1 = np.asarray(W, np.float64).reshape(H, D)
    cl = (W1 * np.asarray(attn_l, np.float64)).sum(1)
    cr = (W1 * np.asarray(attn_r, np.float64)).sum(1)
    prm = np.zeros((P, 8), dtype=np.float32)
    prm[:, 0:4] = cl.astype(np.float32)
    prm[:, 4:8] = cr.astype(np.float32)

    order = np.argsort(dst, kind="stable")
    ss, dd = src[order], dst[order]
    bounds = np.searchsorted(dd, np.arange(NCORES + 1) * nodes_pc)
    in_maps = []
    for k in range(NCORES):
        a, b = bounds[k], bounds[k + 1]
        arrs = _host_prep_core(f, ss[a:b], dd[a:b], k * nodes_pc, pl,
                               packs[k][0], packs[k][1])
        in_maps.append({**arrs, "prm": prm})

    nc = _build_program(pl)
    res = bass_utils.run_bass_kernel_spmd(nc, in_maps,
                                          core_ids=list(range(NCORES)),
                                          trace=False)

    ssum = np.zeros(H, dtype=np.float64)
    for k in range(NCORES):
        raw = res.results[k]["acc"].astype(np.float64)   # [128, nsup*512]
        nsup = raw.shape[1] // BLK
        # p = 32*blk_lo + val (val<8); slot = (sup*3 + blk_lo)*512 + j
        r = raw.reshape(4, 32, nsup, BLK)[:3, :8]          # [3, 8, nsup, 512]
        acc = r.transpose(1, 2, 0, 3).reshape(8, -1)[:, :pl["nblk"] * BLK]
        denom, num = acc[0:4], acc[4:8]
        s = np.where(denom > 0, num / np.maximum(denom, 1e-300), 0.0)
        ssum += s.sum(axis=1)
    sbar = ssum / N
    rbar = sbar[:, None] * W1 + np.asarray(bias_gat, np.float64).reshape(H, D)
    out = rbar.reshape(1, H * D) @ np.asarray(fc_W, np.float64) \
        + np.asarray(fc_b, np.float64)
    return out[0].astype(np.float32)


# revision 16
# speedup vs baseline: 1.0089x; 1.0089x over previous
"""DGL-GAT subgraph encoder kernel for 8 Trainium2 NeuronCores.

With IN_FEATS=1 the GATConv collapses to per-node scalars:
  feat[n,h,d] = f[n]*W1[h,d];  el[n,h] = f[n]*cl[h];  er[n,h] = f[n]*cr[h]
  w[e,h] = exp(lrelu(f[src]*cl[h] + f[dst]*cr[h]))   (softmax max-shift cancels
  in the num/denom ratio; exponents stay < ~25 so no overflow)
  denom[n,h] = seg_sum_dst(w);  num[n,h] = seg_sum_dst(w * f[src])
  s[n,h] = num/denom;  sbar[h] = mean_n s
  out = (sbar[h]*W1[h,:] + bias_gat) @ fc_W + fc_b     (tiny, done on host)

Sharding: core k owns dst nodes [k*12500, (k+1)*12500) and all edges into
them.  Nodes are greedily packed into windows of <=WIN nodes / <=128 edges;
each window's edges form one dst-pure 128-edge column (identical structure
on all 8 cores -> one SPMD program).  Per column the device computes the
per-edge values w, w*fs (DVE z/max + ACT exp, bf16) and an 8-wide one-hot
from the window-local ids (DVE is_equal), then one PE matmul
V[128e,8]^T x onehot[128e,WIN] per column scatters both segment sums into
PSUM ([8,WIN] per window, 3 blocks of 16 windows packed per [128,512] PSUM
supertile at partition offsets 0/32/64).  Supertiles flush via one wide DVE
copy + DMA.  Host decodes the slot-permuted (denom,num) tables; the node
sum is slot-order-invariant, so no inverse permutation is needed (empty
slots have denom=0 and contribute 0).  Measured ~109 us on 8 cores,
rel err ~1e-4 (bf16 edge values, f32 PSUM accumulation).
"""
import numpy as np
import ml_dtypes
import concourse.bass as bass
import concourse.tile as tile
from concourse import bacc, mybir, bass_utils

WIN = 8           # nodes per one-hot window (matmul N)
BLK = 512         # nodes per psum block
P = 128           # edges per column
CHK = 128         # columns per onehot chunk
CCH = 512         # columns per compute/load chunk
NCORES = 8

BF16 = ml_dtypes.bfloat16


def _plan(n_nodes, nwin_max):
    nodes_pc = -(-n_nodes // NCORES)
    ncw = 1
    C = -(-(nwin_max * ncw) // CHK) * CHK
    nblk = ((C - 1) // ncw) // (BLK // WIN) + 1
    return dict(nodes_pc=nodes_pc, nwin=nwin_max, ncw=ncw, C=C, nblk=nblk)


def _pack_windows(deg):
    """Greedy sequential packing: nodes (in order) into windows of <=WIN nodes
    and <=ncw*P edges.  Returns per-node window id and within-window slot."""
    cap = 1 * P
    nodewin = np.empty(len(deg), dtype=np.int64)
    nodeslot = np.empty(len(deg), dtype=np.int64)
    w = nn = ee = 0
    for i, dg in enumerate(deg):
        if nn >= WIN or ee + dg > cap:
            w += 1; nn = 0; ee = 0
        nodewin[i] = w
        nodeslot[i] = nn
        nn += 1; ee += dg
    return nodewin, nodeslot, w + 1


def _host_prep_core(f, src_c, dst_c, lo, pl, nodewin, nodeslot):
    ncw, C = pl["ncw"], pl["C"]
    o = np.argsort(dst_c, kind="stable")
    s_c, d_c = src_c[o], dst_c[o]
    nloc = d_c - lo
    win = nodewin[nloc]
    idl = nodeslot[nloc]
    starts = np.searchsorted(win, np.arange(pl["nwin"]))
    rank = np.arange(len(win)) - starts[win]
    cap = ncw * P
    assert rank.max(initial=0) < cap, "window capacity overflow"
    flat = win * cap + rank

    def scatter(vals, fill, dt):
        a = np.full(C * P, fill, dtype=np.float32)
        a[flat] = vals
        return np.ascontiguousarray(a.reshape(C, P).T).astype(dt)

    return dict(fs=scatter(f[s_c], 0.0, np.float32),
                fd=scatter(f[d_c], 0.0, np.float32),
                ids=scatter(idl.astype(np.float32), -1.0, BF16))


def _build_program(pl):
    C, ncw, nblk = pl["C"], pl["ncw"], pl["nblk"]
    nc = bacc.Bacc("TRN2", target_bir_lowering=False, debug=False,
                   enable_asserts=False, num_devices=NCORES)
    bf = mybir.dt.bfloat16
    f32 = mybir.dt.float32

    fs_d = nc.dram_tensor("fs", [P, C], f32, kind="ExternalInput").ap()
    fd_d = nc.dram_tensor("fd", [P, C], f32, kind="ExternalInput").ap()
    ids_d = nc.dram_tensor("ids", [P, C], bf, kind="ExternalInput").ap()
    prm_d = nc.dram_tensor("prm", [P, 8], f32, kind="ExternalInput").ap()
    nsup = -(-nblk // 3)
    acc_d = nc.dram_tensor("acc", [P, nsup * BLK], f32, kind="ExternalOutput").ap()
    wpb = BLK // WIN

    with tile.TileContext(nc) as tc:
        with tc.tile_pool(name="consts", bufs=1) as cpool, \
             tc.tile_pool(name="io", bufs=4) as io, \
             tc.tile_pool(name="work", bufs=3) as work, \
             tc.tile_pool(name="ohp", bufs=6) as ohp, \
             tc.tile_pool(name="flp", bufs=3) as flp, \
             tc.tile_pool(name="psum", bufs=4, space="PSUM") as psum_p:
            def flush(sup, ps):
                st = flp.tile([P, BLK], f32, tag="fl")
                nc.vector.tensor_copy(st[:], ps[:])
                nc.sync.dma_start(acc_d[:, sup * BLK:(sup + 1) * BLK], st[:])

            prm = cpool.tile([P, 8], f32, name="prm_s")
            nc.sync.dma_start(prm[:], prm_d)
            iota = cpool.tile([P, WIN], mybir.dt.int16, name="iota_s")
            nc.gpsimd.iota(iota[:], pattern=[[1, WIN]], base=0, channel_multiplier=0)
            iotab = cpool.tile([P, WIN], bf, name="iotab_s")
            nc.vector.tensor_copy(iotab[:], iota[:])

            psum_t, cur_blk = None, -1
            chunks = []
            c0x = 0
            while c0x < C:
                chunks.append((c0x, min(CCH, C - c0x)))
                c0x += CCH
            for c0, CL in chunks:
                fst = io.tile([P, CCH], f32, tag="fs")
                fdt = io.tile([P, CCH], f32, tag="fd")
                idst = io.tile([P, CCH], bf, tag="ids")
                fs = fst[:, :CL]; fd = fdt[:, :CL]; ids = idst[:, :CL]
                nc.sync.dma_start(fs, fs_d[:, c0:c0 + CL])
                nc.sync.dma_start(fd, fd_d[:, c0:c0 + CL])
                nc.sync.dma_start(ids, ids_d[:, c0:c0 + CL])

                vi = work.tile([P, 8 * CCH], bf, tag="vi")
                vi3 = vi[:].rearrange("p (v c) -> p v c", v=8)[:, :, :CL]
                t1 = work.tile([P, CCH], f32, tag="t1", name="t1t")[:, :CL]
                z = work.tile([P, CCH], f32, tag="z", name="zt")[:, :CL]
                e1 = work.tile([P, CCH], bf, tag="e1", name="e1t")[:, :CL]
                e2 = work.tile([P, CCH], bf, tag="e2", name="e2t")[:, :CL]
                fsb = work.tile([P, CCH], bf, tag="fsb", name="fsbt")[:, :CL]
                nc.vector.tensor_copy(fsb, fs)
                for h in range(4):
                    nc.vector.tensor_scalar_mul(t1, fd, prm[:, 4 + h:5 + h])
                    nc.vector.scalar_tensor_tensor(
                        out=z, in0=fs, scalar=prm[:, h:h + 1], in1=t1,
                        op0=mybir.AluOpType.mult, op1=mybir.AluOpType.add)
                    nc.scalar.activation(e1, z, mybir.ActivationFunctionType.Exp)
                    nc.scalar.activation(e2, z, mybir.ActivationFunctionType.Exp,
                                         scale=0.2)
                    nc.vector.tensor_tensor(out=vi3[:, h, :], in0=e1, in1=e2,
                                            op=mybir.AluOpType.max)
                    nc.vector.tensor_mul(vi3[:, 4 + h, :], vi3[:, h, :], fsb)

                for ch in range(CL // CHK):
                    t0 = c0 + ch * CHK
                    oh = ohp.tile([P, CHK * WIN], bf, tag="oh")
                    nc.vector.tensor_tensor(
                        out=oh[:].rearrange("p (c w) -> p c w", w=WIN),
                        in0=ids[:, ch * CHK:(ch + 1) * CHK].unsqueeze(-1)
                            .to_broadcast([P, CHK, WIN]),
                        in1=iotab[:].unsqueeze(1).to_broadcast([P, CHK, WIN]),
                        op=mybir.AluOpType.is_equal)
                    for tl in range(CHK):
                        t = t0 + tl
                        w = t // ncw
                        b = w // wpb
                        sup = b // 3
                        if sup != cur_blk:
                            if psum_t is not None:
                                flush(cur_blk, psum_t)
                            psum_t = psum_p.tile([P, BLK], f32, tag="ps")
                            cur_blk = sup
                        wl = w % wpb
                        po = 32 * (b % 3)
                        nc.tensor.matmul(
                            out=psum_t[po:po + 8, wl * WIN:(wl + 1) * WIN],
                            lhsT=vi3[:, :, t - c0],
                            rhs=oh[:, tl * WIN:(tl + 1) * WIN],
                            start=(t % ncw == 0), stop=(t % ncw == ncw - 1))
            flush(cur_blk, psum_t)
    nc.compile()
    return nc


def kernel(features, W, attn_l, attn_r, bias_gat, fc_W, fc_b, src, dst):
    f = np.asarray(features, dtype=np.float32)[:, 0]
    src = np.asarray(src)
    dst = np.asarray(dst)
    N = f.shape[0]
    H, D = np.asarray(attn_l).shape

    nodes_pc = -(-N // NCORES)
    packs = []
    for k in range(NCORES):
        lo = k * nodes_pc
        npc = min(nodes_pc, N - lo)
        deg = np.bincount(dst[(dst >= lo) & (dst < lo + npc)] - lo, minlength=npc)
        packs.append(_pack_windows(deg))
    pl = _plan(N, max(pk[2] for pk in packs))

    W1 = np.asarray(W, np.float64).reshape(H, D)
    cl = (W1 * np.asarray(attn_l, np.float64)).sum(1)
    cr = (W1 * np.asarray(attn_r, np.float64)).sum(1)
    prm = np.zeros((P, 8), dtype=np.float32)
    prm[:, 0:4] = cl.astype(np.float32)
    prm[:, 4:8] = cr.astype(np.float32)

    order = np.argsort(dst, kind="stable")
    ss, dd = src[order], dst[order]
    bounds = np.searchsorted(dd, np.arange(NCORES + 1) * nodes_pc)
    in_maps = []
    for k in range(NCORES):
        a, b = bounds[k], bounds[k + 1]
        arrs = _host_prep_core(f, ss[a:b], dd[a:b], k * nodes_pc, pl,
                               packs[k][0], packs[k][1])
        in_maps.append({**arrs, "prm": prm})

    nc = _build_program(pl)
    res = bass_utils.run_bass_kernel_spmd(nc, in_maps,
                                          core_ids=list(range(NCORES)),
                                          trace=False)

    ssum = np.zeros(H, dtype=np.float64)
    for k in range(NCORES):
        raw = res.results[k]["acc"].astype(np.float64)   # [128, nsup*512]
        nsup = raw.shape[1] // BLK
        # p = 32*blk_lo + val (val<8); slot = (sup*3 + blk_lo)*512 + j
        r = raw.reshape(4, 32, nsup, BLK)[:3, :8]          # [3, 8, nsup, 512]
        acc = r.transpose(1, 2, 0, 3).reshape(8, -1)[:, :pl["nblk"] * BLK]
        denom, num = acc[0:4], acc[4:8]
        s = np.where(denom > 0, num / np.maximum(denom, 1e-300), 0.0)
        ssum += s.sum(axis=1)
    sbar = ssum / N
    rbar = sbar[:, None] * W1 + np.asarray(bias_gat, np.float64).reshape(H, D)
    out = rbar.reshape(1, H * D) @ np.asarray(fc_W, np.float64) \
        + np.asarray(fc_b, np.float64)
    return out[0].astype(np.float32)


# revision 17
# speedup vs baseline: 1.0107x; 1.0018x over previous
"""DGL-GAT subgraph encoder kernel for 8 Trainium2 NeuronCores.

With IN_FEATS=1 the GATConv collapses to per-node scalars:
  feat[n,h,d] = f[n]*W1[h,d];  el[n,h] = f[n]*cl[h];  er[n,h] = f[n]*cr[h]
  w[e,h] = exp(lrelu(f[src]*cl[h] + f[dst]*cr[h]))   (softmax max-shift cancels
  in the num/denom ratio; exponents stay < ~25 so no overflow)
  denom[n,h] = seg_sum_dst(w);  num[n,h] = seg_sum_dst(w * f[src])
  s[n,h] = num/denom;  sbar[h] = mean_n s
  out = (sbar[h]*W1[h,:] + bias_gat) @ fc_W + fc_b     (tiny, done on host)

Sharding: core k owns dst nodes [k*12500, (k+1)*12500) and all edges into
them.  Nodes are greedily packed into windows of <=WIN nodes / <=128 edges;
each window's edges form one dst-pure 128-edge column (identical structure
on all 8 cores -> one SPMD program).  Per column the device computes the
per-edge values w, w*fs (DVE z/max + ACT exp, bf16) and an 8-wide one-hot
from the window-local ids (DVE is_equal), then one PE matmul
V[128e,8]^T x onehot[128e,WIN] per column scatters both segment sums into
PSUM ([8,WIN] per window, 3 blocks of 16 windows packed per [128,512] PSUM
supertile at partition offsets 0/32/64).  Supertiles flush via one wide DVE
copy + DMA.  Host decodes the slot-permuted (denom,num) tables; the node
sum is slot-order-invariant, so no inverse permutation is needed (empty
slots have denom=0 and contribute 0).  Measured ~109 us on 8 cores,
rel err ~1e-4 (bf16 edge values, f32 PSUM accumulation).
"""
import numpy as np
import ml_dtypes
import concourse.bass as bass
import concourse.tile as tile
from concourse import bacc, mybir, bass_utils

WIN = 8           # nodes per one-hot window (matmul N)
BLK = 512         # nodes per psum block
P = 128           # edges per column
CHK = 128         # columns per onehot chunk
CCH = 512         # columns per compute/load chunk
NCORES = 8

BF16 = ml_dtypes.bfloat16


def _plan(n_nodes, nwin_max):
    nodes_pc = -(-n_nodes // NCORES)
    ncw = 1
    C = -(-(nwin_max * ncw) // CHK) * CHK
    nblk = ((C - 1) // ncw) // (BLK // WIN) + 1
    return dict(nodes_pc=nodes_pc, nwin=nwin_max, ncw=ncw, C=C, nblk=nblk)


def _pack_windows(deg):
    """Greedy sequential packing: nodes (in order) into windows of <=WIN nodes
    and <=ncw*P edges.  Returns per-node window id and within-window slot."""
    cap = 1 * P
    nodewin = np.empty(len(deg), dtype=np.int64)
    nodeslot = np.empty(len(deg), dtype=np.int64)
    w = nn = ee = 0
    for i, dg in enumerate(deg):
        if nn >= WIN or ee + dg > cap:
            w += 1; nn = 0; ee = 0
        nodewin[i] = w
        nodeslot[i] = nn
        nn += 1; ee += dg
    return nodewin, nodeslot, w + 1


def _host_prep_core(f, src_c, dst_c, lo, pl, nodewin, nodeslot):
    ncw, C = pl["ncw"], pl["C"]
    o = np.argsort(dst_c, kind="stable")
    s_c, d_c = src_c[o], dst_c[o]
    nloc = d_c - lo
    win = nodewin[nloc]
    idl = nodeslot[nloc]
    starts = np.searchsorted(win, np.arange(pl["nwin"]))
    rank = np.arange(len(win)) - starts[win]
    cap = ncw * P
    assert rank.max(initial=0) < cap, "window capacity overflow"
    flat = win * cap + rank

    def scatter(vals, fill, dt):
        a = np.full(C * P, fill, dtype=np.float32)
        a[flat] = vals
        return np.ascontiguousarray(a.reshape(C, P).T).astype(dt)

    return dict(fs=scatter(f[s_c], 0.0, np.float32),
                fd=scatter(f[d_c], 0.0, np.float32),
                ids=scatter(idl.astype(np.float32), -1.0, BF16))


def _build_program(pl):
    C, ncw, nblk = pl["C"], pl["ncw"], pl["nblk"]
    nc = bacc.Bacc("TRN2", target_bir_lowering=False, debug=False,
                   enable_asserts=False, num_devices=NCORES)
    bf = mybir.dt.bfloat16
    f32 = mybir.dt.float32

    fs_d = nc.dram_tensor("fs", [P, C], f32, kind="ExternalInput").ap()
    fd_d = nc.dram_tensor("fd", [P, C], f32, kind="ExternalInput").ap()
    ids_d = nc.dram_tensor("ids", [P, C], bf, kind="ExternalInput").ap()
    prm_d = nc.dram_tensor("prm", [P, 8], f32, kind="ExternalInput").ap()
    nsup = -(-nblk // 3)
    acc_d = nc.dram_tensor("acc", [P, nsup * BLK], f32, kind="ExternalOutput").ap()
    wpb = BLK // WIN

    with tile.TileContext(nc) as tc:
        with tc.tile_pool(name="consts", bufs=1) as cpool, \
             tc.tile_pool(name="io", bufs=4) as io, \
             tc.tile_pool(name="work", bufs=3) as work, \
             tc.tile_pool(name="ohp", bufs=6) as ohp, \
             tc.tile_pool(name="flp", bufs=3) as flp, \
             tc.tile_pool(name="psum", bufs=4, space="PSUM") as psum_p:
            def flush(sup, ps):
                st = flp.tile([P, BLK], f32, tag="fl")
                nc.vector.tensor_copy(st[:], ps[:])
                nc.sync.dma_start(acc_d[:, sup * BLK:(sup + 1) * BLK], st[:])

            prm = cpool.tile([P, 8], f32, name="prm_s")
            nc.sync.dma_start(prm[:], prm_d)
            iota = cpool.tile([P, WIN], mybir.dt.int16, name="iota_s")
            nc.gpsimd.iota(iota[:], pattern=[[1, WIN]], base=0, channel_multiplier=0)
            iotab = cpool.tile([P, WIN], bf, name="iotab_s")
            nc.vector.tensor_copy(iotab[:], iota[:])

            psum_t, cur_blk = None, -1
            chunks = []
            c0x = 0
            ramp = [128, 128, 256]
            while c0x < C:
                want = ramp.pop(0) if ramp else CCH
                chunks.append((c0x, min(want, C - c0x)))
                c0x += chunks[-1][1]
            for c0, CL in chunks:
                fst = io.tile([P, CCH], f32, tag="fs")
                fdt = io.tile([P, CCH], f32, tag="fd")
                idst = io.tile([P, CCH], bf, tag="ids")
                fs = fst[:, :CL]; fd = fdt[:, :CL]; ids = idst[:, :CL]
                nc.sync.dma_start(fs, fs_d[:, c0:c0 + CL])
                nc.sync.dma_start(fd, fd_d[:, c0:c0 + CL])
                nc.sync.dma_start(ids, ids_d[:, c0:c0 + CL])

                vi = work.tile([P, 8 * CCH], bf, tag="vi")
                vi3 = vi[:].rearrange("p (v c) -> p v c", v=8)[:, :, :CL]
                t1 = work.tile([P, CCH], f32, tag="t1", name="t1t")[:, :CL]
                z = work.tile([P, CCH], f32, tag="z", name="zt")[:, :CL]
                e1 = work.tile([P, CCH], bf, tag="e1", name="e1t")[:, :CL]
                e2 = work.tile([P, CCH], bf, tag="e2", name="e2t")[:, :CL]
                fsb = work.tile([P, CCH], bf, tag="fsb", name="fsbt")[:, :CL]
                nc.vector.tensor_copy(fsb, fs)
                for h in range(4):
                    nc.vector.tensor_scalar_mul(t1, fd, prm[:, 4 + h:5 + h])
                    nc.vector.scalar_tensor_tensor(
                        out=z, in0=fs, scalar=prm[:, h:h + 1], in1=t1,
                        op0=mybir.AluOpType.mult, op1=mybir.AluOpType.add)
                    nc.scalar.activation(e1, z, mybir.ActivationFunctionType.Exp)
                    nc.scalar.activation(e2, z, mybir.ActivationFunctionType.Exp,
                                         scale=0.2)
                    nc.vector.tensor_tensor(out=vi3[:, h, :], in0=e1, in1=e2,
                                            op=mybir.AluOpType.max)
                    nc.vector.tensor_mul(vi3[:, 4 + h, :], vi3[:, h, :], fsb)

                for ch in range(CL // CHK):
                    t0 = c0 + ch * CHK
                    oh = ohp.tile([P, CHK * WIN], bf, tag="oh")
                    nc.vector.tensor_tensor(
                        out=oh[:].rearrange("p (c w) -> p c w", w=WIN),
                        in0=ids[:, ch * CHK:(ch + 1) * CHK].unsqueeze(-1)
                            .to_broadcast([P, CHK, WIN]),
                        in1=iotab[:].unsqueeze(1).to_broadcast([P, CHK, WIN]),
                        op=mybir.AluOpType.is_equal)
                    for tl in range(CHK):
                        t = t0 + tl
                        w = t // ncw
                        b = w // wpb
                        sup = b // 3
                        if sup != cur_blk:
                            if psum_t is not None:
                                flush(cur_blk, psum_t)
                            psum_t = psum_p.tile([P, BLK], f32, tag="ps")
                            cur_blk = sup
                        wl = w % wpb
                        po = 32 * (b % 3)
                        nc.tensor.matmul(
                            out=psum_t[po:po + 8, wl * WIN:(wl + 1) * WIN],
                            lhsT=vi3[:, :, t - c0],
                            rhs=oh[:, tl * WIN:(tl + 1) * WIN],
                            start=(t % ncw == 0), stop=(t % ncw == ncw - 1))
            flush(cur_blk, psum_t)
    nc.compile()
    return nc


def kernel(features, W, attn_l, attn_r, bias_gat, fc_W, fc_b, src, dst):
    f = np.asarray(features, dtype=np.float32)[:, 0]
    src = np.asarray(src)
    dst = np.asarray(dst)
    N = f.shape[0]
    H, D = np.asarray(attn_l).shape

    nodes_pc = -(-N // NCORES)
    packs = []
    for k in range(NCORES):
        lo = k * nodes_pc
        npc = min(nodes_pc, N - lo)
        deg = np.bincount(dst[(dst >= lo) & (dst < lo + npc)] - lo, minlength=npc)
        packs.append(_pack_windows(deg))
    pl = _plan(N, max(pk[2] for pk in packs))

    W1 = np.asarray(W, np.float64).reshape(H, D)
    cl = (W1 * np.asarray(attn_l, np.float64)).sum(1)
    cr = (W1 * np.asarray(attn_r, np.float64)).sum(1)
    prm = np.zeros((P, 8), dtype=np.float32)
    prm[:, 0:4] = cl.astype(np.float32)
    prm[:, 4:8] = cr.astype(np.float32)

    order = np.argsort(dst, kind="stable")
    ss, dd = src[order], dst[order]
    bounds = np.searchsorted(dd, np.arange(NCORES + 1) * nodes_pc)
    in_maps = []
    for k in range(NCORES):
        a, b = bounds[k], bounds[k + 1]
        arrs = _host_prep_core(f, ss[a:b], dd[a:b], k * nodes_pc, pl,
                               packs[k][0], packs[k][1])
        in_maps.append({**arrs, "prm": prm})

    nc = _build_program(pl)
    res = bass_utils.run_bass_kernel_spmd(nc, in_maps,
                                          core_ids=list(range(NCORES)),
                                          trace=False)

    ssum = np.zeros(H, dtype=np.float64)
    for k in range(NCORES):
        raw = res.results[k]["acc"].astype(np.float64)   # [128, nsup*512]
        nsup = raw.shape[1] // BLK
        # p = 32*blk_lo + val (val<8); slot = (sup*3 + blk_lo)*512 + j
        r = raw.reshape(4, 32, nsup, BLK)[:3, :8]          # [3, 8, nsup, 512]
        acc = r.transpose(1, 2, 0, 3).reshape(8, -1)[:, :pl["nblk"] * BLK]
        denom, num = acc[0:4], acc[4:8]
        s = np.where(denom > 0, num / np.maximum(denom, 1e-300), 0.0)
        ssum += s.sum(axis=1)
    sbar = ssum / N
    rbar = sbar[:, None] * W1 + np.asarray(bias_gat, np.float64).reshape(H, D)
    out = rbar.reshape(1, H * D) @ np.asarray(fc_W, np.float64) \
        + np.asarray(fc_b, np.float64)
    return out[0].astype(np.float32)


# revision 18
# speedup vs baseline: 1.2370x; 1.2239x over previous
"""DGL-GAT subgraph encoder kernel for 8 Trainium2 NeuronCores.

With IN_FEATS=1 the GATConv collapses to per-node scalars:
  feat[n,h,d] = f[n]*W1[h,d];  el[n,h] = f[n]*cl[h];  er[n,h] = f[n]*cr[h]
  w[e,h] = exp(lrelu(f[src]*cl[h] + f[dst]*cr[h]))   (softmax max-shift cancels
  in the num/denom ratio; exponents stay < ~25 so no overflow)
  denom[n,h] = seg_sum_dst(w);  num[n,h] = seg_sum_dst(w * f[src])
  s[n,h] = num/denom;  sbar[h] = mean_n s
  out = (sbar[h]*W1[h,:] + bias_gat) @ fc_W + fc_b     (tiny, done on host)

Sharding: core k owns dst nodes [k*12500, (k+1)*12500) and all edges into
them.  Nodes are greedily packed into windows of <=WIN nodes / <=128 edges;
each window's edges form one dst-pure 128-edge column (identical structure
on all 8 cores -> one SPMD program).  Per column the device computes the
per-edge values w, w*fs (DVE z/max + ACT exp, bf16) and an 8-wide one-hot
from the window-local ids (DVE is_equal), then one PE matmul
V[128e,8]^T x onehot[128e,WIN] per column scatters both segment sums into
PSUM ([8,WIN] per window, 3 blocks of 16 windows packed per [128,512] PSUM
supertile at partition offsets 0/32/64).  Supertiles flush via one wide DVE
copy + DMA.  Host decodes the slot-permuted (denom,num) tables; the node
sum is slot-order-invariant, so no inverse permutation is needed (empty
slots have denom=0 and contribute 0).  Measured ~109 us on 8 cores,
rel err ~1e-4 (bf16 edge values, f32 PSUM accumulation).
"""
import numpy as np
import ml_dtypes
import concourse.bass as bass
import concourse.tile as tile
from concourse import bacc, mybir, bass_utils

WIN = 8           # nodes per one-hot window (matmul N)
BLK = 512         # nodes per psum block
P = 128           # edges per column
CHK = 128         # columns per onehot chunk
CCH = 512         # columns per compute/load chunk
NCORES = 8

BF16 = ml_dtypes.bfloat16


def _plan(n_nodes, nwin_max):
    nodes_pc = -(-n_nodes // NCORES)
    ncw = 1
    C = -(-(nwin_max * ncw) // CHK) * CHK
    nblk = ((C - 1) // ncw) // (BLK // WIN) + 1
    return dict(nodes_pc=nodes_pc, nwin=nwin_max, ncw=ncw, C=C, nblk=nblk)


def _pack_windows(deg):
    """Greedy sequential packing: nodes (in order) into windows of <=WIN nodes
    and <=ncw*P edges.  Returns per-node window id and within-window slot."""
    cap = 1 * P
    nodewin = np.empty(len(deg), dtype=np.int64)
    nodeslot = np.empty(len(deg), dtype=np.int64)
    w = nn = ee = 0
    for i, dg in enumerate(deg):
        if nn >= WIN or ee + dg > cap:
            w += 1; nn = 0; ee = 0
        nodewin[i] = w
        nodeslot[i] = nn
        nn += 1; ee += dg
    return nodewin, nodeslot, w + 1


def _host_prep_core(f, src_c, dst_c, lo, pl, nodewin, nodeslot):
    ncw, C = pl["ncw"], pl["C"]
    o = np.argsort(dst_c, kind="stable")
    s_c, d_c = src_c[o], dst_c[o]
    nloc = d_c - lo
    win = nodewin[nloc]
    idl = nodeslot[nloc]
    starts = np.searchsorted(win, np.arange(pl["nwin"]))
    rank = np.arange(len(win)) - starts[win]
    cap = ncw * P
    assert rank.max(initial=0) < cap, "window capacity overflow"
    flat = win * cap + rank

    def scatter(vals, fill, dt):
        a = np.full(C * P, fill, dtype=np.float32)
        a[flat] = vals
        return np.ascontiguousarray(a.reshape(C, P).T).astype(dt)

    return dict(fs=scatter(f[s_c], 0.0, np.float32),
                fd=scatter(f[d_c], 0.0, np.float32),
                ids=scatter(idl.astype(np.float32), -1.0, BF16))


def _build_program(pl):
    C, ncw, nblk = pl["C"], pl["ncw"], pl["nblk"]
    nc = bacc.Bacc("TRN2", target_bir_lowering=False, debug=False,
                   enable_asserts=False, num_devices=NCORES)
    bf = mybir.dt.bfloat16
    f32 = mybir.dt.float32

    fs_d = nc.dram_tensor("fs", [P, C], f32, kind="ExternalInput").ap()
    fd_d = nc.dram_tensor("fd", [P, C], f32, kind="ExternalInput").ap()
    ids_d = nc.dram_tensor("ids", [P, C], bf, kind="ExternalInput").ap()
    prm_d = nc.dram_tensor("prm", [P, 8], f32, kind="ExternalInput").ap()
    nsup = -(-nblk // 3)
    acc_d = nc.dram_tensor("acc", [P, nsup * BLK], f32, kind="ExternalOutput").ap()
    wpb = BLK // WIN

    with tile.TileContext(nc) as tc:
        with tc.tile_pool(name="consts", bufs=1) as cpool, \
             tc.tile_pool(name="io", bufs=4) as io, \
             tc.tile_pool(name="work", bufs=3) as work, \
             tc.tile_pool(name="ohp", bufs=6) as ohp, \
             tc.tile_pool(name="flp", bufs=3) as flp, \
             tc.tile_pool(name="psum", bufs=8, space="PSUM") as psum_p:
            def flush(sup, ps):
                st = flp.tile([P, BLK], f32, tag="fl")
                nc.vector.tensor_copy(st[:], ps[:])
                nc.sync.dma_start(acc_d[:, sup * BLK:(sup + 1) * BLK], st[:])

            prm = cpool.tile([P, 8], f32, name="prm_s")
            nc.sync.dma_start(prm[:], prm_d)
            iota = cpool.tile([P, WIN], mybir.dt.int16, name="iota_s")
            nc.gpsimd.iota(iota[:], pattern=[[1, WIN]], base=0, channel_multiplier=0)
            iotab = cpool.tile([P, WIN], bf, name="iotab_s")
            nc.vector.tensor_copy(iotab[:], iota[:])

            psum_t, cur_blk = None, -1
            pending = []          # [(sup, tile)] awaiting deferred flush
            chunks = []
            c0x = 0
            ramp = [128, 128, 256]
            while c0x < C:
                want = ramp.pop(0) if ramp else CCH
                chunks.append((c0x, min(want, C - c0x)))
                c0x += chunks[-1][1]
            for c0, CL in chunks:
                fst = io.tile([P, CCH], f32, tag="fs")
                fdt = io.tile([P, CCH], f32, tag="fd")
                idst = io.tile([P, CCH], bf, tag="ids")
                fs = fst[:, :CL]; fd = fdt[:, :CL]; ids = idst[:, :CL]
                nc.sync.dma_start(fs, fs_d[:, c0:c0 + CL])
                nc.sync.dma_start(fd, fd_d[:, c0:c0 + CL])
                nc.sync.dma_start(ids, ids_d[:, c0:c0 + CL])

                vi = work.tile([P, 8 * CCH], bf, tag="vi")
                vi3 = vi[:].rearrange("p (v c) -> p v c", v=8)[:, :, :CL]
                t1 = work.tile([P, CCH], f32, tag="t1", name="t1t")[:, :CL]
                z = work.tile([P, CCH], f32, tag="z", name="zt")[:, :CL]
                e1 = work.tile([P, CCH], bf, tag="e1", name="e1t")[:, :CL]
                e2 = work.tile([P, CCH], bf, tag="e2", name="e2t")[:, :CL]
                fsb = work.tile([P, CCH], bf, tag="fsb", name="fsbt")[:, :CL]
                nc.vector.tensor_copy(fsb, fs)
                for h in range(4):
                    nc.vector.tensor_scalar_mul(t1, fd, prm[:, 4 + h:5 + h])
                    nc.vector.scalar_tensor_tensor(
                        out=z, in0=fs, scalar=prm[:, h:h + 1], in1=t1,
                        op0=mybir.AluOpType.mult, op1=mybir.AluOpType.add)
                    nc.scalar.activation(e1, z, mybir.ActivationFunctionType.Exp)
                    nc.scalar.activation(e2, z, mybir.ActivationFunctionType.Exp,
                                         scale=0.2)
                    nc.vector.tensor_tensor(out=vi3[:, h, :], in0=e1, in1=e2,
                                            op=mybir.AluOpType.max)
                    nc.vector.tensor_mul(vi3[:, 4 + h, :], vi3[:, h, :], fsb)

                for ch in range(CL // CHK):
                    t0 = c0 + ch * CHK
                    oh = ohp.tile([P, CHK * WIN], bf, tag="oh")
                    nc.vector.tensor_tensor(
                        out=oh[:].rearrange("p (c w) -> p c w", w=WIN),
                        in0=ids[:, ch * CHK:(ch + 1) * CHK].unsqueeze(-1)
                            .to_broadcast([P, CHK, WIN]),
                        in1=iotab[:].unsqueeze(1).to_broadcast([P, CHK, WIN]),
                        op=mybir.AluOpType.is_equal)
                    for tl in range(CHK):
                        t = t0 + tl
                        w = t // ncw
                        b = w // wpb
                        sup = b // 3
                        if sup != cur_blk:
                            if psum_t is not None:
                                pending.append((cur_blk, psum_t))
                                if len(pending) >= 7:
                                    flush(*pending.pop(0))
                            psum_t = psum_p.tile([P, BLK], f32, tag="ps")
                            cur_blk = sup
                        wl = w % wpb
                        po = 32 * (b % 3)
                        nc.tensor.matmul(
                            out=psum_t[po:po + 8, wl * WIN:(wl + 1) * WIN],
                            lhsT=vi3[:, :, t - c0],
                            rhs=oh[:, tl * WIN:(tl + 1) * WIN],
                            start=(t % ncw == 0), stop=(t % ncw == ncw - 1))
            pending.append((cur_blk, psum_t))
            for sup_ps in pending:
                flush(*sup_ps)
    nc.compile()
    return nc


def kernel(features, W, attn_l, attn_r, bias_gat, fc_W, fc_b, src, dst):
    f = np.asarray(features, dtype=np.float32)[:, 0]
    src = np.asarray(src)
    dst = np.asarray(dst)
    N = f.shape[0]
    H, D = np.asarray(attn_l).shape

    nodes_pc = -(-N // NCORES)
    packs = []
    for k in range(NCORES):
        lo = k * nodes_pc
        npc = min(nodes_pc, N - lo)
        deg = np.bincount(dst[(dst >= lo) & (dst < lo + npc)] - lo, minlength=npc)
        packs.append(_pack_windows(deg))
    pl = _plan(N, max(pk[2] for pk in packs))

    W1 = np.asarray(W, np.float64).reshape(H, D)
    cl = (W1 * np.asarray(attn_l, np.float64)).sum(1)
    cr = (W1 * np.asarray(attn_r, np.float64)).sum(1)
    prm = np.zeros((P, 8), dtype=np.float32)
    prm[:, 0:4] = cl.astype(np.float32)
    prm[:, 4:8] = cr.astype(np.float32)

    order = np.argsort(dst, kind="stable")
    ss, dd = src[order], dst[order]
    bounds = np.searchsorted(dd, np.arange(NCORES + 1) * nodes_pc)
    in_maps = []
    for k in range(NCORES):
        a, b = bounds[k], bounds[k + 1]
        arrs = _host_prep_core(f, ss[a:b], dd[a:b], k * nodes_pc, pl,
                               packs[k][0], packs[k][1])
        in_maps.append({**arrs, "prm": prm})

    nc = _build_program(pl)
    res = bass_utils.run_bass_kernel_spmd(nc, in_maps,
                                          core_ids=list(range(NCORES)),
                                          trace=False)

    ssum = np.zeros(H, dtype=np.float64)
    for k in range(NCORES):
        raw = res.results[k]["acc"].astype(np.float64)   # [128, nsup*512]
        nsup = raw.shape[1] // BLK
        # p = 32*blk_lo + val (val<8); slot = (sup*3 + blk_lo)*512 + j
        r = raw.reshape(4, 32, nsup, BLK)[:3, :8]          # [3, 8, nsup, 512]
        acc = r.transpose(1, 2, 0, 3).reshape(8, -1)[:, :pl["nblk"] * BLK]
        denom, num = acc[0:4], acc[4:8]
        s = np.where(denom > 0, num / np.maximum(denom, 1e-300), 0.0)
        ssum += s.sum(axis=1)
    sbar = ssum / N
    rbar = sbar[:, None] * W1 + np.asarray(bias_gat, np.float64).reshape(H, D)
    out = rbar.reshape(1, H * D) @ np.asarray(fc_W, np.float64) \
        + np.asarray(fc_b, np.float64)
    return out[0].astype(np.float32)


# revision 19
# speedup vs baseline: 1.2406x; 1.0030x over previous
"""DGL-GAT subgraph encoder kernel for 8 Trainium2 NeuronCores.

With IN_FEATS=1 the GATConv collapses to per-node scalars:
  feat[n,h,d] = f[n]*W1[h,d];  el[n,h] = f[n]*cl[h];  er[n,h] = f[n]*cr[h]
  w[e,h] = exp(lrelu(f[src]*cl[h] + f[dst]*cr[h]))   (softmax max-shift cancels
  in the num/denom ratio; exponents stay < ~25 so no overflow)
  denom[n,h] = seg_sum_dst(w);  num[n,h] = seg_sum_dst(w * f[src])
  s[n,h] = num/denom;  sbar[h] = mean_n s
  out = (sbar[h]*W1[h,:] + bias_gat) @ fc_W + fc_b     (tiny, done on host)

Sharding: core k owns dst nodes [k*12500, (k+1)*12500) and all edges into
them.  Nodes are greedily packed into windows of <=WIN nodes / <=128 edges;
each window's edges form one dst-pure 128-edge column (identical structure
on all 8 cores -> one SPMD program).  Per column the device computes the
per-edge values w, w*fs (DVE z/max + ACT exp, bf16) and an 8-wide one-hot
from the window-local ids (DVE is_equal), then one PE matmul
V[128e,8]^T x onehot[128e,WIN] per column scatters both segment sums into
PSUM ([8,WIN] per window, 3 blocks of 16 windows packed per [128,512] PSUM
supertile at partition offsets 0/32/64).  Supertiles flush via one wide DVE
copy + DMA.  Host decodes the slot-permuted (denom,num) tables; the node
sum is slot-order-invariant, so no inverse permutation is needed (empty
slots have denom=0 and contribute 0).  Measured ~109 us on 8 cores,
rel err ~1e-4 (bf16 edge values, f32 PSUM accumulation).
"""
import numpy as np
import ml_dtypes
import concourse.bass as bass
import concourse.tile as tile
from concourse import bacc, mybir, bass_utils

WIN = 8           # nodes per one-hot window (matmul N)
BLK = 512         # nodes per psum block
P = 128           # edges per column
CHK = 128         # columns per onehot chunk
CCH = 512         # columns per compute/load chunk
NCORES = 8

BF16 = ml_dtypes.bfloat16


def _plan(n_nodes, nwin_max):
    nodes_pc = -(-n_nodes // NCORES)
    ncw = 1
    C = -(-(nwin_max * ncw) // CHK) * CHK
    nblk = ((C - 1) // ncw) // (BLK // WIN) + 1
    return dict(nodes_pc=nodes_pc, nwin=nwin_max, ncw=ncw, C=C, nblk=nblk)


def _pack_windows(deg):
    """Greedy sequential packing: nodes (in order) into windows of <=WIN nodes
    and <=ncw*P edges.  Returns per-node window id and within-window slot."""
    cap = 1 * P
    nodewin = np.empty(len(deg), dtype=np.int64)
    nodeslot = np.empty(len(deg), dtype=np.int64)
    w = nn = ee = 0
    for i, dg in enumerate(deg):
        if nn >= WIN or ee + dg > cap:
            w += 1; nn = 0; ee = 0
        nodewin[i] = w
        nodeslot[i] = nn
        nn += 1; ee += dg
    return nodewin, nodeslot, w + 1


def _host_prep_core(f, src_c, dst_c, lo, pl, nodewin, nodeslot):
    ncw, C = pl["ncw"], pl["C"]
    o = np.argsort(dst_c, kind="stable")
    s_c, d_c = src_c[o], dst_c[o]
    nloc = d_c - lo
    win = nodewin[nloc]
    idl = nodeslot[nloc]
    starts = np.searchsorted(win, np.arange(pl["nwin"]))
    rank = np.arange(len(win)) - starts[win]
    cap = ncw * P
    assert rank.max(initial=0) < cap, "window capacity overflow"
    flat = win * cap + rank

    def scatter(vals, fill, dt):
        a = np.full(C * P, fill, dtype=np.float32)
        a[flat] = vals
        return np.ascontiguousarray(a.reshape(C, P).T).astype(dt)

    return dict(fs=scatter(f[s_c], 0.0, np.float32),
                fd=scatter(f[d_c], 0.0, np.float32),
                ids=scatter(idl.astype(np.float32), -1.0, BF16))


def _build_program(pl):
    C, ncw, nblk = pl["C"], pl["ncw"], pl["nblk"]
    nc = bacc.Bacc("TRN2", target_bir_lowering=False, debug=False,
                   enable_asserts=False, num_devices=NCORES)
    bf = mybir.dt.bfloat16
    f32 = mybir.dt.float32

    fs_d = nc.dram_tensor("fs", [P, C], f32, kind="ExternalInput").ap()
    fd_d = nc.dram_tensor("fd", [P, C], f32, kind="ExternalInput").ap()
    ids_d = nc.dram_tensor("ids", [P, C], bf, kind="ExternalInput").ap()
    prm_d = nc.dram_tensor("prm", [P, 8], f32, kind="ExternalInput").ap()
    nsup = -(-nblk // 3)
    acc_d = nc.dram_tensor("acc", [P, nsup * BLK], f32, kind="ExternalOutput").ap()
    wpb = BLK // WIN

    with tile.TileContext(nc) as tc:
        with tc.tile_pool(name="consts", bufs=1) as cpool, \
             tc.tile_pool(name="io", bufs=4) as io, \
             tc.tile_pool(name="work", bufs=3) as work, \
             tc.tile_pool(name="ohp", bufs=6) as ohp, \
             tc.tile_pool(name="flp", bufs=3) as flp, \
             tc.tile_pool(name="psum", bufs=8, space="PSUM") as psum_p:
            def flush(sup, ps):
                st = flp.tile([P, BLK], f32, tag="fl")
                nc.vector.tensor_copy(st[:], ps[:])
                nc.sync.dma_start(acc_d[:, sup * BLK:(sup + 1) * BLK], st[:])

            prm = cpool.tile([P, 8], f32, name="prm_s")
            nc.sync.dma_start(prm[:], prm_d)
            iota = cpool.tile([P, WIN], mybir.dt.int16, name="iota_s")
            nc.gpsimd.iota(iota[:], pattern=[[1, WIN]], base=0, channel_multiplier=0)
            iotab = cpool.tile([P, WIN], bf, name="iotab_s")
            nc.vector.tensor_copy(iotab[:], iota[:])

            psum_t, cur_blk = None, -1
            pending = []          # [(sup, tile)] awaiting deferred flush
            sizes = []
            rem = C - 512          # head ramp 128+128+256
            while rem > 512 + 256:
                sizes.append(CCH)
                rem -= CCH
            tail = []
            while rem > 0:
                t = min(256, rem) if rem > 128 else rem
                tail.append(t)
                rem -= t
            sizes = [128, 128, 256] + sizes + tail
            chunks = []
            c0x = 0
            for want in sizes:
                chunks.append((c0x, want))
                c0x += want
            assert c0x == C, (c0x, C)
            for c0, CL in chunks:
                fst = io.tile([P, CCH], f32, tag="fs")
                fdt = io.tile([P, CCH], f32, tag="fd")
                idst = io.tile([P, CCH], bf, tag="ids")
                fs = fst[:, :CL]; fd = fdt[:, :CL]; ids = idst[:, :CL]
                nc.sync.dma_start(fs, fs_d[:, c0:c0 + CL])
                nc.sync.dma_start(fd, fd_d[:, c0:c0 + CL])
                nc.sync.dma_start(ids, ids_d[:, c0:c0 + CL])

                vi = work.tile([P, 8 * CCH], bf, tag="vi")
                vi3 = vi[:].rearrange("p (v c) -> p v c", v=8)[:, :, :CL]
                t1 = work.tile([P, CCH], f32, tag="t1", name="t1t")[:, :CL]
                z = work.tile([P, CCH], f32, tag="z", name="zt")[:, :CL]
                e1 = work.tile([P, CCH], bf, tag="e1", name="e1t")[:, :CL]
                e2 = work.tile([P, CCH], bf, tag="e2", name="e2t")[:, :CL]
                fsb = work.tile([P, CCH], bf, tag="fsb", name="fsbt")[:, :CL]
                nc.vector.tensor_copy(fsb, fs)
                for h in range(4):
                    nc.vector.tensor_scalar_mul(t1, fd, prm[:, 4 + h:5 + h])
                    nc.vector.scalar_tensor_tensor(
                        out=z, in0=fs, scalar=prm[:, h:h + 1], in1=t1,
                        op0=mybir.AluOpType.mult, op1=mybir.AluOpType.add)
                    nc.scalar.activation(e1, z, mybir.ActivationFunctionType.Exp)
                    nc.scalar.activation(e2, z, mybir.ActivationFunctionType.Exp,
                                         scale=0.2)
                    nc.vector.tensor_tensor(out=vi3[:, h, :], in0=e1, in1=e2,
                                            op=mybir.AluOpType.max)
                    nc.vector.tensor_mul(vi3[:, 4 + h, :], vi3[:, h, :], fsb)

                for ch in range(CL // CHK):
                    t0 = c0 + ch * CHK
                    oh = ohp.tile([P, CHK * WIN], bf, tag="oh")
                    nc.vector.tensor_tensor(
                        out=oh[:].rearrange("p (c w) -> p c w", w=WIN),
                        in0=ids[:, ch * CHK:(ch + 1) * CHK].unsqueeze(-1)
                            .to_broadcast([P, CHK, WIN]),
                        in1=iotab[:].unsqueeze(1).to_broadcast([P, CHK, WIN]),
                        op=mybir.AluOpType.is_equal)
                    for tl in range(CHK):
                        t = t0 + tl
                        w = t // ncw
                        b = w // wpb
                        sup = b // 3
                        if sup != cur_blk:
                            if psum_t is not None:
                                pending.append((cur_blk, psum_t))
                                if len(pending) >= 5:
                                    flush(*pending.pop(0))
                            psum_t = psum_p.tile([P, BLK], f32, tag="ps")
                            cur_blk = sup
                        wl = w % wpb
                        po = 32 * (b % 3)
                        nc.tensor.matmul(
                            out=psum_t[po:po + 8, wl * WIN:(wl + 1) * WIN],
                            lhsT=vi3[:, :, t - c0],
                            rhs=oh[:, tl * WIN:(tl + 1) * WIN],
                            start=(t % ncw == 0), stop=(t % ncw == ncw - 1))
            pending.append((cur_blk, psum_t))
            for sup_ps in pending:
                flush(*sup_ps)
    nc.compile()
    return nc


def kernel(features, W, attn_l, attn_r, bias_gat, fc_W, fc_b, src, dst):
    f = np.asarray(features, dtype=np.float32)[:, 0]
    src = np.asarray(src)
    dst = np.asarray(dst)
    N = f.shape[0]
    H, D = np.asarray(attn_l).shape

    nodes_pc = -(-N // NCORES)
    packs = []
    for k in range(NCORES):
        lo = k * nodes_pc
        npc = min(nodes_pc, N - lo)
        deg = np.bincount(dst[(dst >= lo) & (dst < lo + npc)] - lo, minlength=npc)
        packs.append(_pack_windows(deg))
    pl = _plan(N, max(pk[2] for pk in packs))

    W1 = np.asarray(W, np.float64).reshape(H, D)
    cl = (W1 * np.asarray(attn_l, np.float64)).sum(1)
    cr = (W1 * np.asarray(attn_r, np.float64)).sum(1)
    prm = np.zeros((P, 8), dtype=np.float32)
    prm[:, 0:4] = cl.astype(np.float32)
    prm[:, 4:8] = cr.astype(np.float32)

    order = np.argsort(dst, kind="stable")
    ss, dd = src[order], dst[order]
    bounds = np.searchsorted(dd, np.arange(NCORES + 1) * nodes_pc)
    in_maps = []
    for k in range(NCORES):
        a, b = bounds[k], bounds[k + 1]
        arrs = _host_prep_core(f, ss[a:b], dd[a:b], k * nodes_pc, pl,
                               packs[k][0], packs[k][1])
        in_maps.append({**arrs, "prm": prm})

    nc = _build_program(pl)
    res = bass_utils.run_bass_kernel_spmd(nc, in_maps,
                                          core_ids=list(range(NCORES)),
                                          trace=False)

    ssum = np.zeros(H, dtype=np.float64)
    for k in range(NCORES):
        raw = res.results[k]["acc"].astype(np.float64)   # [128, nsup*512]
        nsup = raw.shape[1] // BLK
        # p = 32*blk_lo + val (val<8); slot = (sup*3 + blk_lo)*512 + j
        r = raw.reshape(4, 32, nsup, BLK)[:3, :8]          # [3, 8, nsup, 512]
        acc = r.transpose(1, 2, 0, 3).reshape(8, -1)[:, :pl["nblk"] * BLK]
        denom, num = acc[0:4], acc[4:8]
        s = np.where(denom > 0, num / np.maximum(denom, 1e-300), 0.0)
        ssum += s.sum(axis=1)
    sbar = ssum / N
    rbar = sbar[:, None] * W1 + np.asarray(bias_gat, np.float64).reshape(H, D)
    out = rbar.reshape(1, H * D) @ np.asarray(fc_W, np.float64) \
        + np.asarray(fc_b, np.float64)
    return out[0].astype(np.float32)


# revision 20
# speedup vs baseline: 1.2784x; 1.0304x over previous
"""DGL-GAT subgraph encoder kernel for 8 Trainium2 NeuronCores.

With IN_FEATS=1 the GATConv collapses to per-node scalars:
  feat[n,h,d] = f[n]*W1[h,d];  el[n,h] = f[n]*cl[h];  er[n,h] = f[n]*cr[h]
  w[e,h] = exp(lrelu(f[src]*cl[h] + f[dst]*cr[h]))   (softmax max-shift cancels
  in the num/denom ratio; exponents stay < ~25 so no overflow)
  denom[n,h] = seg_sum_dst(w);  num[n,h] = seg_sum_dst(w * f[src])
  s[n,h] = num/denom;  sbar[h] = mean_n s
  out = (sbar[h]*W1[h,:] + bias_gat) @ fc_W + fc_b     (tiny, done on host)

Sharding: core k owns dst nodes [k*12500, (k+1)*12500) and all edges into
them.  Nodes are greedily packed into windows of <=WIN nodes / <=128 edges;
each window's edges form one dst-pure 128-edge column (identical structure
on all 8 cores -> one SPMD program).  Per column the device computes the
per-edge values w, w*fs (DVE z/max + ACT exp, bf16) and an 8-wide one-hot
from the window-local ids (DVE is_equal), then one PE matmul
V[128e,8]^T x onehot[128e,WIN] per column scatters both segment sums into
PSUM ([8,WIN] per window, 3 blocks of 16 windows packed per [128,512] PSUM
supertile at partition offsets 0/32/64).  Supertiles flush via one wide DVE
copy + DMA.  Host decodes the slot-permuted (denom,num) tables; the node
sum is slot-order-invariant, so no inverse permutation is needed (empty
slots have denom=0 and contribute 0).  Measured ~109 us on 8 cores,
rel err ~1e-4 (bf16 edge values, f32 PSUM accumulation).
"""
import numpy as np
import ml_dtypes
import concourse.bass as bass
import concourse.tile as tile
from concourse import bacc, mybir, bass_utils

WIN = 8           # nodes per one-hot window (matmul N)
BLK = 512         # nodes per psum block
P = 128           # edges per column
CHK = 128         # columns per onehot chunk
CCH = 512         # columns per compute/load chunk
NCORES = 8

BF16 = ml_dtypes.bfloat16


def _plan(n_nodes, nwin_max):
    nodes_pc = -(-n_nodes // NCORES)
    ncw = 1
    C = -(-(nwin_max * ncw) // CHK) * CHK
    nblk = ((C - 1) // ncw) // (BLK // WIN) + 1
    return dict(nodes_pc=nodes_pc, nwin=nwin_max, ncw=ncw, C=C, nblk=nblk)


def _pack_windows(deg):
    """Balanced packing of nodes into windows of <=WIN nodes / <=P edges:
    snake-deal nodes (sorted by degree desc) across windows, then move nodes
    out of overflowing windows into fresh tail windows."""
    n = len(deg)
    cap = P
    nwins = max(-(-n // WIN), -(-int(deg.sum()) // (cap - 4)))
    idx = np.argsort(-deg, kind="stable")
    pad = WIN * nwins - n
    snake = np.concatenate([idx, np.full(pad, -1, np.int64)]).reshape(WIN, nwins)
    snake[1::2] = snake[1::2, ::-1]
    nodewin = np.empty(n, dtype=np.int64)
    nodeslot = np.empty(n, dtype=np.int64)
    for r in range(WIN):
        row = snake[r]
        m = row >= 0
        nodewin[row[m]] = np.nonzero(m)[0]
        nodeslot[row[m]] = r
    loads = np.bincount(nodewin, weights=deg, minlength=nwins).astype(np.int64)
    counts = np.bincount(nodewin, minlength=nwins)
    # fix overflows: strip smallest nodes from over-cap windows into a spill
    spill = []
    order_in_win = [[] for _ in range(nwins)]
    for i in range(n):
        order_in_win[nodewin[i]].append(i)
    for wdx in np.nonzero(loads > cap)[0]:
        members = sorted(order_in_win[wdx], key=lambda i: deg[i])
        j = 0
        while loads[wdx] > cap:
            i = members[j]; j += 1
            loads[wdx] -= deg[i]
            counts[wdx] -= 1
            spill.append(i)
    # re-pack spill greedily into fresh windows
    w = nwins - 1
    nn = WIN
    ee = cap
    for i in sorted(spill, key=lambda i: -deg[i]):
        if nn >= WIN or ee + deg[i] > cap:
            w += 1; nn = 0; ee = 0
        nodewin[i] = w
        nodeslot[i] = nn
        nn += 1; ee += deg[i]
    nwins_tot = w + 1
    # re-derive slots within each window to be unique 0..count-1
    o = np.lexsort((nodeslot, nodewin))
    st = np.searchsorted(nodewin[o], np.arange(nwins_tot))
    nodeslot[o] = np.arange(n) - st[nodewin[o]]
    assert np.bincount(nodewin, weights=deg).max() <= cap
    assert np.bincount(nodewin).max() <= WIN
    return nodewin, nodeslot, nwins_tot


def _host_prep_core(f, src_c, dst_c, lo, pl, nodewin, nodeslot):
    ncw, C = pl["ncw"], pl["C"]
    nloc0 = dst_c - lo
    win0 = nodewin[nloc0]
    o = np.argsort(win0, kind="stable")
    s_c, d_c = src_c[o], dst_c[o]
    nloc = d_c - lo
    win = win0[o]
    idl = nodeslot[nloc]
    starts = np.searchsorted(win, np.arange(pl["nwin"]))
    rank = np.arange(len(win)) - starts[win]
    cap = ncw * P
    assert rank.max(initial=0) < cap, "window capacity overflow"
    flat = win * cap + rank

    def scatter(vals, fill, dt):
        a = np.full(C * P, fill, dtype=np.float32)
        a[flat] = vals
        return np.ascontiguousarray(a.reshape(C, P).T).astype(dt)

    return dict(fs=scatter(f[s_c], 0.0, np.float32),
                fd=scatter(f[d_c], 0.0, np.float32),
                ids=scatter(idl.astype(np.float32), -1.0, BF16))


def _build_program(pl):
    C, ncw, nblk = pl["C"], pl["ncw"], pl["nblk"]
    nc = bacc.Bacc("TRN2", target_bir_lowering=False, debug=False,
                   enable_asserts=False, num_devices=NCORES)
    bf = mybir.dt.bfloat16
    f32 = mybir.dt.float32

    fs_d = nc.dram_tensor("fs", [P, C], f32, kind="ExternalInput").ap()
    fd_d = nc.dram_tensor("fd", [P, C], f32, kind="ExternalInput").ap()
    ids_d = nc.dram_tensor("ids", [P, C], bf, kind="ExternalInput").ap()
    prm_d = nc.dram_tensor("prm", [P, 8], f32, kind="ExternalInput").ap()
    nsup = -(-nblk // 3)
    acc_d = nc.dram_tensor("acc", [P, nsup * BLK], f32, kind="ExternalOutput").ap()
    wpb = BLK // WIN

    with tile.TileContext(nc) as tc:
        with tc.tile_pool(name="consts", bufs=1) as cpool, \
             tc.tile_pool(name="io", bufs=4) as io, \
             tc.tile_pool(name="work", bufs=3) as work, \
             tc.tile_pool(name="ohp", bufs=6) as ohp, \
             tc.tile_pool(name="flp", bufs=3) as flp, \
             tc.tile_pool(name="psum", bufs=8, space="PSUM") as psum_p:
            def flush(sup, ps):
                st = flp.tile([P, BLK], f32, tag="fl")
                nc.vector.tensor_copy(st[:], ps[:])
                nc.sync.dma_start(acc_d[:, sup * BLK:(sup + 1) * BLK], st[:])

            prm = cpool.tile([P, 8], f32, name="prm_s")
            nc.sync.dma_start(prm[:], prm_d)
            iota = cpool.tile([P, WIN], mybir.dt.int16, name="iota_s")
            nc.gpsimd.iota(iota[:], pattern=[[1, WIN]], base=0, channel_multiplier=0)
            iotab = cpool.tile([P, WIN], bf, name="iotab_s")
            nc.vector.tensor_copy(iotab[:], iota[:])

            psum_t, cur_blk = None, -1
            pending = []          # [(sup, tile)] awaiting deferred flush
            sizes = []
            rem = C - 512          # head ramp 128+128+256
            while rem > 512 + 256:
                sizes.append(CCH)
                rem -= CCH
            tail = []
            while rem > 0:
                t = min(256, rem) if rem > 128 else rem
                tail.append(t)
                rem -= t
            sizes = [128, 128, 256] + sizes + tail
            chunks = []
            c0x = 0
            for want in sizes:
                chunks.append((c0x, want))
                c0x += want
            assert c0x == C, (c0x, C)
            for c0, CL in chunks:
                fst = io.tile([P, CCH], f32, tag="fs")
                fdt = io.tile([P, CCH], f32, tag="fd")
                idst = io.tile([P, CCH], bf, tag="ids")
                fs = fst[:, :CL]; fd = fdt[:, :CL]; ids = idst[:, :CL]
                nc.sync.dma_start(fs, fs_d[:, c0:c0 + CL])
                nc.scalar.dma_start(fd, fd_d[:, c0:c0 + CL])
                nc.scalar.dma_start(ids, ids_d[:, c0:c0 + CL])

                vi = work.tile([P, 8 * CCH], bf, tag="vi")
                vi3 = vi[:].rearrange("p (v c) -> p v c", v=8)[:, :, :CL]
                t1 = work.tile([P, CCH], f32, tag="t1", name="t1t")[:, :CL]
                z = work.tile([P, CCH], f32, tag="z", name="zt")[:, :CL]
                e1 = work.tile([P, CCH], bf, tag="e1", name="e1t")[:, :CL]
                e2 = work.tile([P, CCH], bf, tag="e2", name="e2t")[:, :CL]
                fsb = work.tile([P, CCH], bf, tag="fsb", name="fsbt")[:, :CL]
                nc.vector.tensor_copy(fsb, fs)
                for h in range(4):
                    nc.vector.tensor_scalar_mul(t1, fd, prm[:, 4 + h:5 + h])
                    nc.vector.scalar_tensor_tensor(
                        out=z, in0=fs, scalar=prm[:, h:h + 1], in1=t1,
                        op0=mybir.AluOpType.mult, op1=mybir.AluOpType.add)
                    nc.scalar.activation(e1, z, mybir.ActivationFunctionType.Exp)
                    nc.scalar.activation(e2, z, mybir.ActivationFunctionType.Exp,
                                         scale=0.2)
                    nc.vector.tensor_tensor(out=vi3[:, h, :], in0=e1, in1=e2,
                                            op=mybir.AluOpType.max)
                    nc.vector.tensor_mul(vi3[:, 4 + h, :], vi3[:, h, :], fsb)

                for ch in range(CL // CHK):
                    t0 = c0 + ch * CHK
                    oh = ohp.tile([P, CHK * WIN], bf, tag="oh")
                    nc.vector.tensor_tensor(
                        out=oh[:].rearrange("p (c w) -> p c w", w=WIN),
                        in0=ids[:, ch * CHK:(ch + 1) * CHK].unsqueeze(-1)
                            .to_broadcast([P, CHK, WIN]),
                        in1=iotab[:].unsqueeze(1).to_broadcast([P, CHK, WIN]),
                        op=mybir.AluOpType.is_equal)
                    for tl in range(CHK):
                        t = t0 + tl
                        w = t // ncw
                        b = w // wpb
                        sup = b // 3
                        if sup != cur_blk:
                            if psum_t is not None:
                                pending.append((cur_blk, psum_t))
                                if len(pending) >= 5:
                                    flush(*pending.pop(0))
                            psum_t = psum_p.tile([P, BLK], f32, tag="ps")
                            cur_blk = sup
                        wl = w % wpb
                        po = 32 * (b % 3)
                        nc.tensor.matmul(
                            out=psum_t[po:po + 8, wl * WIN:(wl + 1) * WIN],
                            lhsT=vi3[:, :, t - c0],
                            rhs=oh[:, tl * WIN:(tl + 1) * WIN],
                            start=(t % ncw == 0), stop=(t % ncw == ncw - 1))
            pending.append((cur_blk, psum_t))
            for sup_ps in pending:
                flush(*sup_ps)
    nc.compile()
    return nc


def kernel(features, W, attn_l, attn_r, bias_gat, fc_W, fc_b, src, dst):
    f = np.asarray(features, dtype=np.float32)[:, 0]
    src = np.asarray(src)
    dst = np.asarray(dst)
    N = f.shape[0]
    H, D = np.asarray(attn_l).shape

    nodes_pc = -(-N // NCORES)
    packs = []
    for k in range(NCORES):
        lo = k * nodes_pc
        npc = min(nodes_pc, N - lo)
        deg = np.bincount(dst[(dst >= lo) & (dst < lo + npc)] - lo, minlength=npc)
        packs.append(_pack_windows(deg))
    pl = _plan(N, max(pk[2] for pk in packs))

    W1 = np.asarray(W, np.float64).reshape(H, D)
    cl = (W1 * np.asarray(attn_l, np.float64)).sum(1)
    cr = (W1 * np.asarray(attn_r, np.float64)).sum(1)
    prm = np.zeros((P, 8), dtype=np.float32)
    prm[:, 0:4] = cl.astype(np.float32)
    prm[:, 4:8] = cr.astype(np.float32)

    order = np.argsort(dst, kind="stable")
    ss, dd = src[order], dst[order]
    bounds = np.searchsorted(dd, np.arange(NCORES + 1) * nodes_pc)
    in_maps = []
    for k in range(NCORES):
        a, b = bounds[k], bounds[k + 1]
        arrs = _host_prep_core(f, ss[a:b], dd[a:b], k * nodes_pc, pl,
                               packs[k][0], packs[k][1])
        in_maps.append({**arrs, "prm": prm})

    nc = _build_program(pl)
    res = bass_utils.run_bass_kernel_spmd(nc, in_maps,
                                          core_ids=list(range(NCORES)),
                                          trace=False)

    ssum = np.zeros(H, dtype=np.float64)
    for k in range(NCORES):
        raw = res.results[k]["acc"].astype(np.float64)   # [128, nsup*512]
        nsup = raw.shape[1] // BLK
        # p = 32*blk_lo + val (val<8); slot = (sup*3 + blk_lo)*512 + j
        r = raw.reshape(4, 32, nsup, BLK)[:3, :8]          # [3, 8, nsup, 512]
        acc = r.transpose(1, 2, 0, 3).reshape(8, -1)[:, :pl["nblk"] * BLK]
        denom, num = acc[0:4], acc[4:8]
        s = np.where(denom > 0, num / np.maximum(denom, 1e-300), 0.0)
        ssum += s.sum(axis=1)
    sbar = ssum / N
    rbar = sbar[:, None] * W1 + np.asarray(bias_gat, np.float64).reshape(H, D)
    out = rbar.reshape(1, H * D) @ np.asarray(fc_W, np.float64) \
        + np.asarray(fc_b, np.float64)
    return out[0].astype(np.float32)


# revision 21
# speedup vs baseline: 1.2834x; 1.0039x over previous
"""DGL-GAT subgraph encoder kernel for 8 Trainium2 NeuronCores.

With IN_FEATS=1 the GATConv collapses to per-node scalars:
  feat[n,h,d] = f[n]*W1[h,d];  el[n,h] = f[n]*cl[h];  er[n,h] = f[n]*cr[h]
  w[e,h] = exp(lrelu(f[src]*cl[h] + f[dst]*cr[h]))   (softmax max-shift cancels
  in the num/denom ratio; exponents stay < ~25 so no overflow)
  denom[n,h] = seg_sum_dst(w);  num[n,h] = seg_sum_dst(w * f[src])
  s[n,h] = num/denom;  sbar[h] = mean_n s
  out = (sbar[h]*W1[h,:] + bias_gat) @ fc_W + fc_b     (tiny, done on host)

Sharding: core k owns dst nodes [k*12500, (k+1)*12500) and all edges into
them.  Nodes are greedily packed into windows of <=WIN nodes / <=128 edges;
each window's edges form one dst-pure 128-edge column (identical structure
on all 8 cores -> one SPMD program).  Per column the device computes the
per-edge values w, w*fs (DVE z/max + ACT exp, bf16) and an 8-wide one-hot
from the window-local ids (DVE is_equal), then one PE matmul
V[128e,8]^T x onehot[128e,WIN] per column scatters both segment sums into
PSUM ([8,WIN] per window, 3 blocks of 16 windows packed per [128,512] PSUM
supertile at partition offsets 0/32/64).  Supertiles flush via one wide DVE
copy + DMA.  Host decodes the slot-permuted (denom,num) tables; the node
sum is slot-order-invariant, so no inverse permutation is needed (empty
slots have denom=0 and contribute 0).  Measured ~109 us on 8 cores,
rel err ~1e-4 (bf16 edge values, f32 PSUM accumulation).
"""
import numpy as np
import ml_dtypes
import concourse.bass as bass
import concourse.tile as tile
from concourse import bacc, mybir, bass_utils

WIN = 8           # nodes per one-hot window (matmul N)
BLK = 512         # nodes per psum block
P = 128           # edges per column
CHK = 128         # columns per onehot chunk
CCH = 512         # columns per compute/load chunk
NCORES = 8

BF16 = ml_dtypes.bfloat16


def _plan(n_nodes, nwin_max):
    nodes_pc = -(-n_nodes // NCORES)
    ncw = 1
    C = -(-(nwin_max * ncw) // CHK) * CHK
    nblk = ((C - 1) // ncw) // (BLK // WIN) + 1
    return dict(nodes_pc=nodes_pc, nwin=nwin_max, ncw=ncw, C=C, nblk=nblk)


def _pack_windows(deg):
    """Balanced packing of nodes into windows of <=WIN nodes / <=P edges:
    snake-deal nodes (sorted by degree desc) across windows, then move nodes
    out of overflowing windows into fresh tail windows."""
    n = len(deg)
    cap = P
    nwins = max(-(-n // WIN), -(-int(deg.sum()) // (cap - 4)))
    idx = np.argsort(-deg, kind="stable")
    pad = WIN * nwins - n
    snake = np.concatenate([idx, np.full(pad, -1, np.int64)]).reshape(WIN, nwins)
    snake[1::2] = snake[1::2, ::-1]
    nodewin = np.empty(n, dtype=np.int64)
    nodeslot = np.empty(n, dtype=np.int64)
    for r in range(WIN):
        row = snake[r]
        m = row >= 0
        nodewin[row[m]] = np.nonzero(m)[0]
        nodeslot[row[m]] = r
    loads = np.bincount(nodewin, weights=deg, minlength=nwins).astype(np.int64)
    counts = np.bincount(nodewin, minlength=nwins)
    # fix overflows: strip smallest nodes from over-cap windows into a spill
    spill = []
    order_in_win = [[] for _ in range(nwins)]
    for i in range(n):
        order_in_win[nodewin[i]].append(i)
    for wdx in np.nonzero(loads > cap)[0]:
        members = sorted(order_in_win[wdx], key=lambda i: deg[i])
        j = 0
        while loads[wdx] > cap:
            i = members[j]; j += 1
            loads[wdx] -= deg[i]
            counts[wdx] -= 1
            spill.append(i)
    # re-pack spill greedily into fresh windows
    w = nwins - 1
    nn = WIN
    ee = cap
    for i in sorted(spill, key=lambda i: -deg[i]):
        if nn >= WIN or ee + deg[i] > cap:
            w += 1; nn = 0; ee = 0
        nodewin[i] = w
        nodeslot[i] = nn
        nn += 1; ee += deg[i]
    nwins_tot = w + 1
    # re-derive slots within each window to be unique 0..count-1
    o = np.lexsort((nodeslot, nodewin))
    st = np.searchsorted(nodewin[o], np.arange(nwins_tot))
    nodeslot[o] = np.arange(n) - st[nodewin[o]]
    assert np.bincount(nodewin, weights=deg).max() <= cap
    assert np.bincount(nodewin).max() <= WIN
    return nodewin, nodeslot, nwins_tot


def _host_prep_core(f, src_c, dst_c, lo, pl, nodewin, nodeslot):
    ncw, C = pl["ncw"], pl["C"]
    nloc0 = dst_c - lo
    win0 = nodewin[nloc0]
    o = np.argsort(win0, kind="stable")
    s_c, d_c = src_c[o], dst_c[o]
    nloc = d_c - lo
    win = win0[o]
    idl = nodeslot[nloc]
    starts = np.searchsorted(win, np.arange(pl["nwin"]))
    rank = np.arange(len(win)) - starts[win]
    cap = ncw * P
    assert rank.max(initial=0) < cap, "window capacity overflow"
    flat = win * cap + rank

    def scatter(vals, fill, dt):
        a = np.full(C * P, fill, dtype=np.float32)
        a[flat] = vals
        return np.ascontiguousarray(a.reshape(C, P).T).astype(dt)

    return dict(fs=scatter(f[s_c], 0.0, np.float32),
                fd=scatter(f[d_c], 0.0, np.float32),
                ids=scatter(idl.astype(np.float32), -1.0, BF16))


def _build_program(pl):
    C, ncw, nblk = pl["C"], pl["ncw"], pl["nblk"]
    nc = bacc.Bacc("TRN2", target_bir_lowering=False, debug=False,
                   enable_asserts=False, num_devices=NCORES)
    bf = mybir.dt.bfloat16
    f32 = mybir.dt.float32

    fs_d = nc.dram_tensor("fs", [P, C], f32, kind="ExternalInput").ap()
    fd_d = nc.dram_tensor("fd", [P, C], f32, kind="ExternalInput").ap()
    ids_d = nc.dram_tensor("ids", [P, C], bf, kind="ExternalInput").ap()
    prm_d = nc.dram_tensor("prm", [P, 8], f32, kind="ExternalInput").ap()
    nsup = -(-nblk // 3)
    acc_d = nc.dram_tensor("acc", [P, nsup * BLK], f32, kind="ExternalOutput").ap()
    wpb = BLK // WIN

    with tile.TileContext(nc) as tc:
        with tc.tile_pool(name="consts", bufs=1) as cpool, \
             tc.tile_pool(name="io", bufs=4) as io, \
             tc.tile_pool(name="work", bufs=3) as work, \
             tc.tile_pool(name="ohp", bufs=6) as ohp, \
             tc.tile_pool(name="flp", bufs=3) as flp, \
             tc.tile_pool(name="psum", bufs=8, space="PSUM") as psum_p:
            def flush(sup, ps):
                st = flp.tile([P, BLK], f32, tag="fl")
                nc.vector.tensor_copy(st[:], ps[:])
                nc.sync.dma_start(acc_d[:, sup * BLK:(sup + 1) * BLK], st[:])

            prm = cpool.tile([P, 8], f32, name="prm_s")
            nc.sync.dma_start(prm[:], prm_d)
            iota = cpool.tile([P, WIN], mybir.dt.int16, name="iota_s")
            nc.gpsimd.iota(iota[:], pattern=[[1, WIN]], base=0, channel_multiplier=0)
            iotab = cpool.tile([P, WIN], bf, name="iotab_s")
            nc.vector.tensor_copy(iotab[:], iota[:])

            psum_t, cur_blk = None, -1
            pending = []          # [(sup, tile)] awaiting deferred flush
            sizes = []
            rem = C - 512          # head ramp 128+128+256
            while rem > 512 + 256:
                sizes.append(CCH)
                rem -= CCH
            tail = []
            while rem > 0:
                t = min(256, rem) if rem > 128 else rem
                tail.append(t)
                rem -= t
            sizes = [128, 128, 256] + sizes + tail
            chunks = []
            c0x = 0
            for want in sizes:
                chunks.append((c0x, want))
                c0x += want
            assert c0x == C, (c0x, C)
            loaded = {}

            def emit_loads(ci):
                c0, CL = chunks[ci]
                fst = io.tile([P, CCH], f32, tag="fs", name="fst")
                fdt = io.tile([P, CCH], f32, tag="fd", name="fdt")
                idst = io.tile([P, CCH], bf, tag="ids", name="idst")
                fs = fst[:, :CL]; fd = fdt[:, :CL]; ids = idst[:, :CL]
                nc.sync.dma_start(fs, fs_d[:, c0:c0 + CL])
                nc.scalar.dma_start(fd, fd_d[:, c0:c0 + CL])
                nc.sync.dma_start(ids, ids_d[:, c0:c0 + CL])
                loaded[ci] = (fs, fd, ids)

            for cj in range(min(3, len(chunks))):
                emit_loads(cj)
            for ci, (c0, CL) in enumerate(chunks):
                if ci + 3 < len(chunks):
                    emit_loads(ci + 3)
                fs, fd, ids = loaded.pop(ci)

                vi = work.tile([P, 8 * CCH], bf, tag="vi")
                vi3 = vi[:].rearrange("p (v c) -> p v c", v=8)[:, :, :CL]
                t1 = work.tile([P, CCH], f32, tag="t1", name="t1t")[:, :CL]
                z = work.tile([P, CCH], f32, tag="z", name="zt")[:, :CL]
                e1 = work.tile([P, CCH], bf, tag="e1", name="e1t")[:, :CL]
                e2 = work.tile([P, CCH], bf, tag="e2", name="e2t")[:, :CL]
                fsb = work.tile([P, CCH], bf, tag="fsb", name="fsbt")[:, :CL]
                nc.vector.tensor_copy(fsb, fs)
                for h in range(4):
                    nc.vector.tensor_scalar_mul(t1, fd, prm[:, 4 + h:5 + h])
                    nc.vector.scalar_tensor_tensor(
                        out=z, in0=fs, scalar=prm[:, h:h + 1], in1=t1,
                        op0=mybir.AluOpType.mult, op1=mybir.AluOpType.add)
                    nc.scalar.activation(e1, z, mybir.ActivationFunctionType.Exp)
                    nc.scalar.activation(e2, z, mybir.ActivationFunctionType.Exp,
                                         scale=0.2)
                    nc.vector.tensor_tensor(out=vi3[:, h, :], in0=e1, in1=e2,
                                            op=mybir.AluOpType.max)
                    nc.vector.tensor_mul(vi3[:, 4 + h, :], vi3[:, h, :], fsb)

                for ch in range(CL // CHK):
                    t0 = c0 + ch * CHK
                    oh = ohp.tile([P, CHK * WIN], bf, tag="oh")
                    nc.vector.tensor_tensor(
                        out=oh[:].rearrange("p (c w) -> p c w", w=WIN),
                        in0=ids[:, ch * CHK:(ch + 1) * CHK].unsqueeze(-1)
                            .to_broadcast([P, CHK, WIN]),
                        in1=iotab[:].unsqueeze(1).to_broadcast([P, CHK, WIN]),
                        op=mybir.AluOpType.is_equal)
                    for tl in range(CHK):
                        t = t0 + tl
                        w = t // ncw
                        b = w // wpb
                        sup = b // 3
                        if sup != cur_blk:
                            if psum_t is not None:
                                pending.append((cur_blk, psum_t))
                                if len(pending) >= 5:
                                    flush(*pending.pop(0))
                            psum_t = psum_p.tile([P, BLK], f32, tag="ps")
                            cur_blk = sup
                        wl = w % wpb
                        po = 32 * (b % 3)
                        nc.tensor.matmul(
                            out=psum_t[po:po + 8, wl * WIN:(wl + 1) * WIN],
                            lhsT=vi3[:, :, t - c0],
                            rhs=oh[:, tl * WIN:(tl + 1) * WIN],
                            start=(t % ncw == 0), stop=(t % ncw == ncw - 1))
            pending.append((cur_blk, psum_t))
            for sup_ps in pending:
                flush(*sup_ps)
    nc.compile()
    return nc


def kernel(features, W, attn_l, attn_r, bias_gat, fc_W, fc_b, src, dst):
    f = np.asarray(features, dtype=np.float32)[:, 0]
    src = np.asarray(src)
    dst = np.asarray(dst)
    N = f.shape[0]
    H, D = np.asarray(attn_l).shape

    nodes_pc = -(-N // NCORES)
    packs = []
    for k in range(NCORES):
        lo = k * nodes_pc
        npc = min(nodes_pc, N - lo)
        deg = np.bincount(dst[(dst >= lo) & (dst < lo + npc)] - lo, minlength=npc)
        packs.append(_pack_windows(deg))
    pl = _plan(N, max(pk[2] for pk in packs))

    W1 = np.asarray(W, np.float64).reshape(H, D)
    cl = (W1 * np.asarray(attn_l, np.float64)).sum(1)
    cr = (W1 * np.asarray(attn_r, np.float64)).sum(1)
    prm = np.zeros((P, 8), dtype=np.float32)
    prm[:, 0:4] = cl.astype(np.float32)
    prm[:, 4:8] = cr.astype(np.float32)

    order = np.argsort(dst, kind="stable")
    ss, dd = src[order], dst[order]
    bounds = np.searchsorted(dd, np.arange(NCORES + 1) * nodes_pc)
    in_maps = []
    for k in range(NCORES):
        a, b = bounds[k], bounds[k + 1]
        arrs = _host_prep_core(f, ss[a:b], dd[a:b], k * nodes_pc, pl,
                               packs[k][0], packs[k][1])
        in_maps.append({**arrs, "prm": prm})

    nc = _build_program(pl)
    res = bass_utils.run_bass_kernel_spmd(nc, in_maps,
                                          core_ids=list(range(NCORES)),
                                          trace=False)

    ssum = np.zeros(H, dtype=np.float64)
    for k in range(NCORES):
        raw = res.results[k]["acc"].astype(np.float64)   # [128, nsup*512]
        nsup = raw.shape[1] // BLK
        # p = 32*blk_lo + val (val<8); slot = (sup*3 + blk_lo)*512 + j
        r = raw.reshape(4, 32, nsup, BLK)[:3, :8]          # [3, 8, nsup, 512]
        acc = r.transpose(1, 2, 0, 3).reshape(8, -1)[:, :pl["nblk"] * BLK]
        denom, num = acc[0:4], acc[4:8]
        s = np.where(denom > 0, num / np.maximum(denom, 1e-300), 0.0)
        ssum += s.sum(axis=1)
    sbar = ssum / N
    rbar = sbar[:, None] * W1 + np.asarray(bias_gat, np.float64).reshape(H, D)
    out = rbar.reshape(1, H * D) @ np.asarray(fc_W, np.float64) \
        + np.asarray(fc_b, np.float64)
    return out[0].astype(np.float32)


# revision 22
# speedup vs baseline: 1.2888x; 1.0042x over previous
"""DGL-GAT subgraph encoder kernel for 8 Trainium2 NeuronCores.

With IN_FEATS=1 the GATConv collapses to per-node scalars:
  feat[n,h,d] = f[n]*W1[h,d];  el[n,h] = f[n]*cl[h];  er[n,h] = f[n]*cr[h]
  w[e,h] = exp(lrelu(f[src]*cl[h] + f[dst]*cr[h]))   (softmax max-shift cancels
  in the num/denom ratio; exponents stay < ~25 so no overflow)
  denom[n,h] = seg_sum_dst(w);  num[n,h] = seg_sum_dst(w * f[src])
  s[n,h] = num/denom;  sbar[h] = mean_n s
  out = (sbar[h]*W1[h,:] + bias_gat) @ fc_W + fc_b     (tiny, done on host)

Sharding: core k owns dst nodes [k*12500, (k+1)*12500) and all edges into
them.  Nodes are greedily packed into windows of <=WIN nodes / <=128 edges;
each window's edges form one dst-pure 128-edge column (identical structure
on all 8 cores -> one SPMD program).  Per column the device computes the
per-edge values w, w*fs (DVE z/max + ACT exp, bf16) and an 8-wide one-hot
from the window-local ids (DVE is_equal), then one PE matmul
V[128e,8]^T x onehot[128e,WIN] per column scatters both segment sums into
PSUM ([8,WIN] per window, 3 blocks of 16 windows packed per [128,512] PSUM
supertile at partition offsets 0/32/64).  Supertiles flush via one wide DVE
copy + DMA.  Host decodes the slot-permuted (denom,num) tables; the node
sum is slot-order-invariant, so no inverse permutation is needed (empty
slots have denom=0 and contribute 0).  Measured ~109 us on 8 cores,
rel err ~1e-4 (bf16 edge values, f32 PSUM accumulation).
"""
import numpy as np
import ml_dtypes
import concourse.bass as bass
import concourse.tile as tile
from concourse import bacc, mybir, bass_utils

WIN = 8           # nodes per one-hot window (matmul N)
BLK = 512         # nodes per psum block
P = 128           # edges per column
CHK = 128         # columns per onehot chunk
CCH = 512         # columns per compute/load chunk
NCORES = 8

BF16 = ml_dtypes.bfloat16


def _plan(n_nodes, nwin_max):
    nodes_pc = -(-n_nodes // NCORES)
    ncw = 1
    C = -(-(nwin_max * ncw) // CHK) * CHK
    nblk = ((C - 1) // ncw) // (BLK // WIN) + 1
    return dict(nodes_pc=nodes_pc, nwin=nwin_max, ncw=ncw, C=C, nblk=nblk)


def _pack_windows(deg):
    """Balanced packing of nodes into windows of <=WIN nodes / <=P edges:
    snake-deal nodes (sorted by degree desc) across windows, then move nodes
    out of overflowing windows into fresh tail windows."""
    n = len(deg)
    cap = P
    nwins = max(-(-n // WIN), -(-int(deg.sum()) // (cap - 4)))
    idx = np.argsort(-deg, kind="stable")
    pad = WIN * nwins - n
    snake = np.concatenate([idx, np.full(pad, -1, np.int64)]).reshape(WIN, nwins)
    snake[1::2] = snake[1::2, ::-1]
    nodewin = np.empty(n, dtype=np.int64)
    nodeslot = np.empty(n, dtype=np.int64)
    for r in range(WIN):
        row = snake[r]
        m = row >= 0
        nodewin[row[m]] = np.nonzero(m)[0]
        nodeslot[row[m]] = r
    loads = np.bincount(nodewin, weights=deg, minlength=nwins).astype(np.int64)
    counts = np.bincount(nodewin, minlength=nwins)
    # fix overflows: strip smallest nodes from over-cap windows into a spill
    spill = []
    order_in_win = [[] for _ in range(nwins)]
    for i in range(n):
        order_in_win[nodewin[i]].append(i)
    for wdx in np.nonzero(loads > cap)[0]:
        members = sorted(order_in_win[wdx], key=lambda i: deg[i])
        j = 0
        while loads[wdx] > cap:
            i = members[j]; j += 1
            loads[wdx] -= deg[i]
            counts[wdx] -= 1
            spill.append(i)
    # re-pack spill greedily into fresh windows
    w = nwins - 1
    nn = WIN
    ee = cap
    for i in sorted(spill, key=lambda i: -deg[i]):
        if nn >= WIN or ee + deg[i] > cap:
            w += 1; nn = 0; ee = 0
        nodewin[i] = w
        nodeslot[i] = nn
        nn += 1; ee += deg[i]
    nwins_tot = w + 1
    # re-derive slots within each window to be unique 0..count-1
    o = np.lexsort((nodeslot, nodewin))
    st = np.searchsorted(nodewin[o], np.arange(nwins_tot))
    nodeslot[o] = np.arange(n) - st[nodewin[o]]
    assert np.bincount(nodewin, weights=deg).max() <= cap
    assert np.bincount(nodewin).max() <= WIN
    return nodewin, nodeslot, nwins_tot


def _host_prep_core(f, src_c, dst_c, lo, pl, nodewin, nodeslot):
    ncw, C = pl["ncw"], pl["C"]
    nloc0 = dst_c - lo
    win0 = nodewin[nloc0]
    o = np.argsort(win0, kind="stable")
    s_c, d_c = src_c[o], dst_c[o]
    nloc = d_c - lo
    win = win0[o]
    idl = nodeslot[nloc]
    starts = np.searchsorted(win, np.arange(pl["nwin"]))
    rank = np.arange(len(win)) - starts[win]
    cap = ncw * P
    assert rank.max(initial=0) < cap, "window capacity overflow"
    flat = win * cap + rank

    def scatter(vals, fill, dt):
        a = np.full(C * P, fill, dtype=np.float32)
        a[flat] = vals
        return np.ascontiguousarray(a.reshape(C, P).T).astype(dt)

    return dict(fs=scatter(f[s_c], 0.0, np.float32),
                fd=scatter(f[d_c], 0.0, np.float32),
                ids=scatter(idl.astype(np.float32), -1.0, BF16))


def _build_program(pl):
    C, ncw, nblk = pl["C"], pl["ncw"], pl["nblk"]
    nc = bacc.Bacc("TRN2", target_bir_lowering=False, debug=False,
                   enable_asserts=False, num_devices=NCORES)
    bf = mybir.dt.bfloat16
    f32 = mybir.dt.float32

    fs_d = nc.dram_tensor("fs", [P, C], f32, kind="ExternalInput").ap()
    fd_d = nc.dram_tensor("fd", [P, C], f32, kind="ExternalInput").ap()
    ids_d = nc.dram_tensor("ids", [P, C], bf, kind="ExternalInput").ap()
    prm_d = nc.dram_tensor("prm", [P, 8], f32, kind="ExternalInput").ap()
    nsup = -(-nblk // 3)
    acc_d = nc.dram_tensor("acc", [P, nsup * BLK], f32, kind="ExternalOutput").ap()
    wpb = BLK // WIN

    with tile.TileContext(nc) as tc:
        with tc.tile_pool(name="consts", bufs=1) as cpool, \
             tc.tile_pool(name="io", bufs=7) as io, \
             tc.tile_pool(name="work", bufs=3) as work, \
             tc.tile_pool(name="ohp", bufs=6) as ohp, \
             tc.tile_pool(name="flp", bufs=3) as flp, \
             tc.tile_pool(name="psum", bufs=8, space="PSUM") as psum_p:
            def flush(sup, ps):
                st = flp.tile([P, BLK], f32, tag="fl")
                nc.vector.tensor_copy(st[:], ps[:])
                nc.sync.dma_start(acc_d[:, sup * BLK:(sup + 1) * BLK], st[:])

            prm = cpool.tile([P, 8], f32, name="prm_s")
            nc.sync.dma_start(prm[:], prm_d)
            iota = cpool.tile([P, WIN], mybir.dt.int16, name="iota_s")
            nc.gpsimd.iota(iota[:], pattern=[[1, WIN]], base=0, channel_multiplier=0)
            iotab = cpool.tile([P, WIN], bf, name="iotab_s")
            nc.vector.tensor_copy(iotab[:], iota[:])

            psum_t, cur_blk = None, -1
            pending = []          # [(sup, tile)] awaiting deferred flush
            sizes = []
            rem = C - 512          # head ramp 128+128+256
            while rem > 512 + 256:
                sizes.append(CCH)
                rem -= CCH
            tail = []
            while rem > 0:
                t = min(256, rem) if rem > 128 else rem
                tail.append(t)
                rem -= t
            sizes = [128, 128, 256] + sizes + tail
            chunks = []
            c0x = 0
            for want in sizes:
                chunks.append((c0x, want))
                c0x += want
            assert c0x == C, (c0x, C)
            loaded = {}

            def emit_loads(ci):
                c0, CL = chunks[ci]
                fst = io.tile([P, CCH], f32, tag="fs", name="fst")
                fdt = io.tile([P, CCH], f32, tag="fd", name="fdt")
                idst = io.tile([P, CCH], bf, tag="ids", name="idst")
                fs = fst[:, :CL]; fd = fdt[:, :CL]; ids = idst[:, :CL]
                nc.sync.dma_start(fs, fs_d[:, c0:c0 + CL])
                nc.scalar.dma_start(fd, fd_d[:, c0:c0 + CL])
                nc.sync.dma_start(ids, ids_d[:, c0:c0 + CL])
                loaded[ci] = (fs, fd, ids)

            for cj in range(min(6, len(chunks))):
                emit_loads(cj)
            for ci, (c0, CL) in enumerate(chunks):
                if ci + 6 < len(chunks):
                    emit_loads(ci + 6)
                fs, fd, ids = loaded.pop(ci)

                vi = work.tile([P, 8 * CCH], bf, tag="vi")
                vi3 = vi[:].rearrange("p (v c) -> p v c", v=8)[:, :, :CL]
                t1 = work.tile([P, CCH], f32, tag="t1", name="t1t")[:, :CL]
                z = work.tile([P, CCH], f32, tag="z", name="zt")[:, :CL]
                e1 = work.tile([P, CCH], bf, tag="e1", name="e1t")[:, :CL]
                e2 = work.tile([P, CCH], bf, tag="e2", name="e2t")[:, :CL]
                fsb = work.tile([P, CCH], bf, tag="fsb", name="fsbt")[:, :CL]
                nc.vector.tensor_copy(fsb, fs)
                for h in range(4):
                    nc.vector.tensor_scalar_mul(t1, fd, prm[:, 4 + h:5 + h])
                    nc.vector.scalar_tensor_tensor(
                        out=z, in0=fs, scalar=prm[:, h:h + 1], in1=t1,
                        op0=mybir.AluOpType.mult, op1=mybir.AluOpType.add)
                    nc.scalar.activation(e1, z, mybir.ActivationFunctionType.Exp)
                    nc.scalar.activation(e2, z, mybir.ActivationFunctionType.Exp,
                                         scale=0.2)
                    nc.vector.tensor_tensor(out=vi3[:, h, :], in0=e1, in1=e2,
                                            op=mybir.AluOpType.max)
                    nc.vector.tensor_mul(vi3[:, 4 + h, :], vi3[:, h, :], fsb)

                for ch in range(CL // CHK):
                    t0 = c0 + ch * CHK
                    oh = ohp.tile([P, CHK * WIN], bf, tag="oh")
                    nc.vector.tensor_tensor(
                        out=oh[:].rearrange("p (c w) -> p c w", w=WIN),
                        in0=ids[:, ch * CHK:(ch + 1) * CHK].unsqueeze(-1)
                            .to_broadcast([P, CHK, WIN]),
                        in1=iotab[:].unsqueeze(1).to_broadcast([P, CHK, WIN]),
                        op=mybir.AluOpType.is_equal)
                    for tl in range(CHK):
                        t = t0 + tl
                        w = t // ncw
                        b = w // wpb
                        sup = b // 3
                        if sup != cur_blk:
                            if psum_t is not None:
                                pending.append((cur_blk, psum_t))
                                if len(pending) >= 5:
                                    flush(*pending.pop(0))
                            psum_t = psum_p.tile([P, BLK], f32, tag="ps")
                            cur_blk = sup
                        wl = w % wpb
                        po = 32 * (b % 3)
                        nc.tensor.matmul(
                            out=psum_t[po:po + 8, wl * WIN:(wl + 1) * WIN],
                            lhsT=vi3[:, :, t - c0],
                            rhs=oh[:, tl * WIN:(tl + 1) * WIN],
                            start=(t % ncw == 0), stop=(t % ncw == ncw - 1))
            pending.append((cur_blk, psum_t))
            for sup_ps in pending:
                flush(*sup_ps)
    nc.compile()
    return nc


def kernel(features, W, attn_l, attn_r, bias_gat, fc_W, fc_b, src, dst):
    f = np.asarray(features, dtype=np.float32)[:, 0]
    src = np.asarray(src)
    dst = np.asarray(dst)
    N = f.shape[0]
    H, D = np.asarray(attn_l).shape

    nodes_pc = -(-N // NCORES)
    packs = []
    for k in range(NCORES):
        lo = k * nodes_pc
        npc = min(nodes_pc, N - lo)
        deg = np.bincount(dst[(dst >= lo) & (dst < lo + npc)] - lo, minlength=npc)
        packs.append(_pack_windows(deg))
    pl = _plan(N, max(pk[2] for pk in packs))

    W1 = np.asarray(W, np.float64).reshape(H, D)
    cl = (W1 * np.asarray(attn_l, np.float64)).sum(1)
    cr = (W1 * np.asarray(attn_r, np.float64)).sum(1)
    prm = np.zeros((P, 8), dtype=np.float32)
    prm[:, 0:4] = cl.astype(np.float32)
    prm[:, 4:8] = cr.astype(np.float32)

    order = np.argsort(dst, kind="stable")
    ss, dd = src[order], dst[order]
    bounds = np.searchsorted(dd, np.arange(NCORES + 1) * nodes_pc)
    in_maps = []
    for k in range(NCORES):
        a, b = bounds[k], bounds[k + 1]
        arrs = _host_prep_core(f, ss[a:b], dd[a:b], k * nodes_pc, pl,
                               packs[k][0], packs[k][1])
        in_maps.append({**arrs, "prm": prm})

    nc = _build_program(pl)
    res = bass_utils.run_bass_kernel_spmd(nc, in_maps,
                                          core_ids=list(range(NCORES)),
                                          trace=False)

    ssum = np.zeros(H, dtype=np.float64)
    for k in range(NCORES):
        raw = res.results[k]["acc"].astype(np.float64)   # [128, nsup*512]
        nsup = raw.shape[1] // BLK
        # p = 32*blk_lo + val (val<8); slot = (sup*3 + blk_lo)*512 + j
        r = raw.reshape(4, 32, nsup, BLK)[:3, :8]          # [3, 8, nsup, 512]
        acc = r.transpose(1, 2, 0, 3).reshape(8, -1)[:, :pl["nblk"] * BLK]
        denom, num = acc[0:4], acc[4:8]
        s = np.where(denom > 0, num / np.maximum(denom, 1e-300), 0.0)
        ssum += s.sum(axis=1)
    sbar = ssum / N
    rbar = sbar[:, None] * W1 + np.asarray(bias_gat, np.float64).reshape(H, D)
    out = rbar.reshape(1, H * D) @ np.asarray(fc_W, np.float64) \
        + np.asarray(fc_b, np.float64)
    return out[0].astype(np.float32)
